# revision 9
# baseline (speedup 1.0000x reference)
"""Trainium2 Bass kernel for a pre-norm transformer block (causal MHA + FFN).

Sharding: pure data-parallel over batch B=128 across 8 NeuronCores
(16 batches/core). No collectives. Each core runs the full block on its
shard: LN1 -> QKV -> causal attention -> Wo + residual -> LN2 -> FFN ->
residual.

Device-side layout strategy (per core, 4096 tokens):
  - LayerNorm computed token-major ([128 tokens, 384] tiles, bn_stats), with
    rstd = exp(-0.5*ln(var+eps)) so ScalarE stays on one ACT table set (Exp).
  - LN gain g is folded into the weight matrices on the host; LN bias be
    becomes a bias vector folded into projection biases (exact).
  - Normalized activations are PE-transposed to feature-major [c, t] tiles,
    which serve as matmul operands for all projections (float32r => 1
    cycle/row at free-dim >= 256).
  - Attention computes scores TRANSPOSED: scoresT[s,t] per (batch, head),
    exp on ScalarE -> probs in bf16, causal mask via gpsimd.affine_select,
    row sums via a ones-vector matmul, attnV token-major with bf16 probs,
    normalization via per-partition tensor_scalar during PSUM->SBUF copy.
  - All projection biases are applied either per-partition during PSUM->SBUF
    copies (free) or as K=1 rank-1 matmul accumulation chunks.
"""

import sys

for _p in ("/opt/trn_rl_repo",):
    if _p not in sys.path:
        sys.path.append(_p)

import numpy as np

import concourse.bass as bass
import concourse.mybir as mybir
import concourse.tile as tile
from concourse import bacc
from concourse.bass_utils import run_bass_kernel_spmd

B, T, C, H, HS = 128, 256, 384, 6, 64
DFF = 4 * C
EPS = 1e-5
NCORES = 8
BL = B // NCORES          # batches per core
NTOK = BL * T             # tokens per core
P = 128
CK = C // P               # channel chunks (3)
FK = DFF // P             # ffn chunks (12)
TT = T // P               # token tiles per batch (2)

F32 = mybir.dt.float32
F32R = mybir.dt.float32r
BF16 = mybir.dt.bfloat16
AF = mybir.ActivationFunctionType
ALU = mybir.AluOpType


def _layernorm_to_feature_major(nc, tc, pools, x_tt, dst_T, identity, eps_tile):
    """LN a token-major [128, C] tile and write normalized transpose into
    dst_T[:, k, tt*128:(tt+1)*128] feature-major slices (pure (x-mu)*rstd;
    gain/bias are folded into downstream weights)."""
    small, ps_tr = pools["small"], pools["ps_tr"]
    for tt in range(TT):
        xt = x_tt[:, tt, :]
        stats = small.tile([P, 6], F32, tag="stats")
        nc.vector.bn_stats(out=stats, in_=xt)
        mv = small.tile([P, 2], F32, tag="mv")
        nc.vector.bn_aggr(out=mv, in_=stats)
        # rstd = exp(-0.5 * ln(var + eps)): keeps ScalarE on the Exp table set
        lnv = small.tile([P, 1], F32, tag="lnv")
        nc.scalar.activation(out=lnv, in_=mv[:, 1:2], func=AF.Ln, bias=eps_tile)
        rstd = small.tile([P, 1], F32, tag="rstd")
        nc.scalar.activation(out=rstd, in_=lnv, func=AF.Exp, scale=-0.5)
        hn = pools["hn"].tile([P, C], F32R, tag="hn")
        nc.vector.tensor_scalar(
            out=hn, in0=xt, scalar1=mv[:, 0:1], scalar2=rstd,
            op0=ALU.subtract, op1=ALU.mult,
        )
        for k in range(CK):
            ps = ps_tr.tile([P, P], F32R, tag="tr")
            nc.tensor.transpose(ps, hn[:, k * P:(k + 1) * P], identity)
            nc.scalar.activation(
                out=dst_T[:, k, tt * P:(tt + 1) * P], in_=ps.bitcast(F32),
                func=AF.Copy)


def build_transformer(nc):
    """Emit the whole per-core transformer graph under a TileContext."""
    xs = nc.dram_tensor("xs", [NTOK, C], F32, kind="ExternalInput").ap()
    wq = nc.dram_tensor("wq", [CK, P, C], F32R, kind="ExternalInput").ap()
    wk = nc.dram_tensor("wk", [CK, P, C], F32R, kind="ExternalInput").ap()
    wv = nc.dram_tensor("wv", [CK, P, C], F32R, kind="ExternalInput").ap()
    wo = nc.dram_tensor("wo", [CK, P, C], F32R, kind="ExternalInput").ap()
    w1 = nc.dram_tensor("w1", [CK, P, DFF], F32R, kind="ExternalInput").ap()
    w2 = nc.dram_tensor("w2", [FK, P, C], F32R, kind="ExternalInput").ap()
    bq = nc.dram_tensor("bq", [C], F32, kind="ExternalInput").ap()
    bk = nc.dram_tensor("bk", [C], F32, kind="ExternalInput").ap()
    bv = nc.dram_tensor("bv", [C], F32R, kind="ExternalInput").ap()
    bo = nc.dram_tensor("bo", [C], F32R, kind="ExternalInput").ap()
    b1 = nc.dram_tensor("b1", [DFF], F32, kind="ExternalInput").ap()
    b2 = nc.dram_tensor("b2", [C], F32R, kind="ExternalInput").ap()
    ident = nc.dram_tensor("ident", [P, P], F32R, kind="ExternalInput").ap()
    onesr = nc.dram_tensor("onesr", [1, P], F32R, kind="ExternalInput").ap()
    out = nc.dram_tensor("out", [NTOK, C], F32, kind="ExternalOutput").ap()

    from contextlib import ExitStack
    with tile.TileContext(nc) as tc, ExitStack() as ctx:
        const = ctx.enter_context(tc.tile_pool(name="const", bufs=1))
        io_pool = ctx.enter_context(tc.tile_pool(name="io", bufs=2))
        act_pool = ctx.enter_context(tc.tile_pool(name="act", bufs=2))
        hn_pool = ctx.enter_context(tc.tile_pool(name="hn", bufs=3))
        wei_pool = ctx.enter_context(tc.tile_pool(name="wei", bufs=4))
        small = ctx.enter_context(tc.tile_pool(name="small", bufs=6))
        rc_pool = ctx.enter_context(tc.tile_pool(name="rc", bufs=26))
        ps_tr = ctx.enter_context(tc.tile_pool(name="ps_tr", bufs=1, space="PSUM"))
        ps_mm = ctx.enter_context(tc.tile_pool(name="ps_mm", bufs=2, space="PSUM"))
        ps_big = ctx.enter_context(tc.tile_pool(name="ps_big", bufs=2, space="PSUM"))
        ps_attn = ctx.enter_context(tc.tile_pool(name="ps_attn", bufs=2, space="PSUM"))
        ps_sum = ctx.enter_context(tc.tile_pool(name="ps_sum", bufs=1, space="PSUM"))
        pools = {"small": small, "hn": hn_pool, "ps_tr": ps_tr}

        # ---- persistent constants ----
        wq_sb = const.tile([P, CK, C], F32R)
        nc.sync.dma_start(out=wq_sb, in_=wq.rearrange("k p d -> p k d"))
        wk_sb = const.tile([P, CK, C], F32R)
        nc.sync.dma_start(out=wk_sb, in_=wk.rearrange("k p d -> p k d"))
        wv_sb = const.tile([P, CK, C], F32R)
        nc.sync.dma_start(out=wv_sb, in_=wv.rearrange("k p d -> p k d"))
        wo_sb = const.tile([P, CK, C], F32R)
        nc.sync.dma_start(out=wo_sb, in_=wo.rearrange("k p d -> p k d"))
        w1_sb = const.tile([P, CK, DFF], F32R)
        nc.sync.dma_start(out=w1_sb, in_=w1.rearrange("k p d -> p k d"))
        w2_sb = const.tile([P, FK, C], F32R)
        nc.sync.dma_start(out=w2_sb, in_=w2.rearrange("k p d -> p k d"))
        bq_sb = const.tile([P, CK], F32)
        nc.sync.dma_start(out=bq_sb, in_=bq.rearrange("(k p) -> p k", p=P))
        bk_sb = const.tile([P, CK], F32)
        nc.sync.dma_start(out=bk_sb, in_=bk.rearrange("(k p) -> p k", p=P))
        b1_sb = const.tile([P, FK], F32)
        nc.sync.dma_start(out=b1_sb, in_=b1.rearrange("(f p) -> p f", p=P))
        bv_row = const.tile([1, C], F32R)
        nc.sync.dma_start(out=bv_row, in_=bv.rearrange("(a d) -> a d", a=1))
        bo_row = const.tile([1, C], F32R)
        nc.sync.dma_start(out=bo_row, in_=bo.rearrange("(a d) -> a d", a=1))
        b2_row = const.tile([1, C], F32R)
        nc.sync.dma_start(out=b2_row, in_=b2.rearrange("(a d) -> a d", a=1))
        ones1 = const.tile([1, P], F32R)
        nc.sync.dma_start(out=ones1, in_=onesr)
        ones_bf = const.tile([P, 1], BF16)
        nc.vector.memset(ones_bf, 1.0)
        identity = const.tile([P, P], F32R)
        nc.sync.dma_start(out=identity, in_=ident)
        eps_tile = const.tile([P, 1], F32)
        nc.vector.memset(eps_tile, EPS)

        for b in range(BL):
            # ---- load x (both token tiles) ----
            xa = io_pool.tile([P, TT, C], F32, tag="xa")
            nc.sync.dma_start(
                out=xa, in_=xs[b * T:(b + 1) * T, :].rearrange("(tt p) c -> p tt c", p=P))

            # ---- LN1 -> h1T feature-major [c, t] ----
            h1T = act_pool.tile([P, CK, T], F32R, tag="h1T")
            _layernorm_to_feature_major(nc, tc, pools, xa, h1T, identity, eps_tile)

            # ---- QKV projections ----
            # Q^T, K^T feature-major [d, t]; V token-major [t, d] in bf16
            QT = act_pool.tile([P, CK, T], F32R, tag="QT")
            KT = act_pool.tile([P, CK, T], F32R, tag="KT")
            for m in range(CK):
                psq = ps_mm.tile([P, T], F32, tag="mm")
                for k in range(CK):
                    nc.tensor.matmul(
                        psq, wq_sb[:, k, m * P:(m + 1) * P],
                        h1T[:, k, :], start=(k == 0), stop=(k == CK - 1))
                nc.scalar.activation(
                    out=QT[:, m, :], in_=psq, func=AF.Identity, bias=bq_sb[:, m:m + 1])
                psk = ps_mm.tile([P, T], F32, tag="mm")
                for k in range(CK):
                    nc.tensor.matmul(
                        psk, wk_sb[:, k, m * P:(m + 1) * P],
                        h1T[:, k, :], start=(k == 0), stop=(k == CK - 1))
                nc.scalar.activation(
                    out=KT[:, m, :], in_=psk, func=AF.Identity, bias=bk_sb[:, m:m + 1])
            Vtm = act_pool.tile([P, TT, C], BF16, tag="Vtm")
            for tt in range(TT):
                psv = ps_big.tile([P, C], F32, tag="big")
                for k in range(CK):
                    nc.tensor.matmul(
                        psv, h1T[:, k, tt * P:(tt + 1) * P],
                        wv_sb[:, k, :], start=(k == 0), stop=False)
                nc.tensor.matmul(
                    psv, ones1, bv_row,
                    start=False, stop=True)
                nc.vector.tensor_copy(out=Vtm[:, tt, :], in_=psv)

            # ---- attention (per head): scoresT -> exp -> mask -> rowsum/attnV
            attn_ps = [
                ps_attn.tile([P, C], F32, tag="attn", name=f"attn_ps{b}_{tt}")
                for tt in range(TT)
            ]
            rcs = {}
            for h in range(H):
                po = 64 * (h % 2)
                m = h // 2
                qh = QT[po:po + HS, m, :]
                kh = KT[po:po + HS, m, :]
                weiT = wei_pool.tile([P, TT, T], BF16, tag="weiT")
                for sc in range(TT):
                    pss = ps_mm.tile([P, T], F32, tag="mm")
                    nc.tensor.matmul(
                        pss, kh[:, sc * P:(sc + 1) * P],
                        qh, start=True, stop=True)
                    if sc == 0:
                        nc.scalar.activation(
                            out=weiT[:, 0, :], in_=pss, func=AF.Exp, scale=HS ** -0.5)
                    else:
                        nc.scalar.activation(
                            out=weiT[:, 1, P:], in_=pss[:, P:], func=AF.Exp,
                            scale=HS ** -0.5)
                # causal mask on the two diagonal blocks: keep where t >= s
                nc.gpsimd.affine_select(
                    out=weiT[:, 0, 0:P], in_=weiT[:, 0, 0:P],
                    compare_op=ALU.is_ge, fill=0.0, base=0,
                    pattern=[[1, P]], channel_multiplier=-1)
                nc.gpsimd.affine_select(
                    out=weiT[:, 1, P:], in_=weiT[:, 1, P:],
                    compare_op=ALU.is_ge, fill=0.0, base=0,
                    pattern=[[1, P]], channel_multiplier=-1)
                # row sums (over s) via ones-vector matmuls; then 1/sum
                for tt in range(TT):
                    psr = ps_sum.tile([P, 1], F32, tag="sum")
                    if tt == 0:
                        nc.tensor.matmul(psr, weiT[:, 0, 0:P], ones_bf,
                                         start=True, stop=True)
                    else:
                        nc.tensor.matmul(psr, weiT[:, 0, P:], ones_bf,
                                         start=True, stop=False)
                        nc.tensor.matmul(psr, weiT[:, 1, P:], ones_bf,
                                         start=False, stop=True)
                    rc = rc_pool.tile([P, 1], F32, tag="rc")
                    nc.vector.reciprocal(out=rc, in_=psr)
                    rcs[(h, tt)] = rc
                # attnV token-major into per-head column slice of attn psum
                for tt in range(TT):
                    dst = attn_ps[tt][:, h * HS:(h + 1) * HS]
                    if tt == 0:
                        nc.tensor.matmul(dst, weiT[:, 0, 0:P],
                                         Vtm[:, 0, h * HS:(h + 1) * HS],
                                         start=True, stop=True)
                    else:
                        nc.tensor.matmul(dst, weiT[:, 0, P:],
                                         Vtm[:, 0, h * HS:(h + 1) * HS],
                                         start=True, stop=False)
                        nc.tensor.matmul(dst, weiT[:, 1, P:],
                                         Vtm[:, 1, h * HS:(h + 1) * HS],
                                         start=False, stop=True)

            # ---- normalize heads, transpose to feature-major ----
            attn_T = act_pool.tile([P, CK, T], F32R, tag="attnT")
            for tt in range(TT):
                attn_sb = hn_pool.tile([P, C], F32R, tag="attn_sb")
                for h in range(H):
                    nc.vector.tensor_scalar_mul(
                        out=attn_sb[:, h * HS:(h + 1) * HS],
                        in0=attn_ps[tt][:, h * HS:(h + 1) * HS],
                        scalar1=rcs[(h, tt)])
                for k in range(CK):
                    pst = ps_tr.tile([P, P], F32R, tag="tr")
                    nc.tensor.transpose(
                        pst, attn_sb[:, k * P:(k + 1) * P], identity)
                    nc.scalar.activation(
                        out=attn_T[:, k, tt * P:(tt + 1) * P], in_=pst.bitcast(F32),
                        func=AF.Copy)

            # ---- Wo projection + residual -> xmid (token-major) ----
            xmid = io_pool.tile([P, TT, C], F32, tag="xmid")
            for tt in range(TT):
                pso = ps_big.tile([P, C], F32, tag="big")
                for k in range(CK):
                    nc.tensor.matmul(
                        pso, attn_T[:, k, tt * P:(tt + 1) * P],
                        wo_sb[:, k, :], start=(k == 0), stop=False)
                nc.tensor.matmul(pso, ones1, bo_row,
                                 start=False, stop=True)
                nc.vector.tensor_add(out=xmid[:, tt, :], in0=xa[:, tt, :], in1=pso)

            # ---- LN2 -> h2T; FFN ----
            h2T = act_pool.tile([P, CK, T], F32R, tag="h2T")
            _layernorm_to_feature_major(nc, tc, pools, xmid, h2T, identity, eps_tile)

            ff1T = act_pool.tile([P, FK, T], F32R, tag="ff1T")
            for f in range(FK):
                psf = ps_mm.tile([P, T], F32, tag="mm")
                for k in range(CK):
                    nc.tensor.matmul(
                        psf, w1_sb[:, k, f * P:(f + 1) * P],
                        h2T[:, k, :], start=(k == 0), stop=(k == CK - 1))
                nc.scalar.activation(
                    out=ff1T[:, f, :], in_=psf, func=AF.Relu, bias=b1_sb[:, f:f + 1])
            for tt in range(TT):
                ps2 = ps_big.tile([P, C], F32, tag="big")
                for f in range(FK):
                    nc.tensor.matmul(
                        ps2, ff1T[:, f, tt * P:(tt + 1) * P],
                        w2_sb[:, f, :], start=(f == 0), stop=False)
                nc.tensor.matmul(ps2, ones1, b2_row,
                                 start=False, stop=True)
                yt = io_pool.tile([P, C], F32, tag="yt")
                nc.vector.tensor_add(out=yt, in0=xmid[:, tt, :], in1=ps2)
                nc.sync.dma_start(
                    out=out[b * T + tt * P: b * T + (tt + 1) * P, :], in_=yt)
    return nc


_NC_CACHE = {}


def get_nc():
    if "nc" not in _NC_CACHE:
        nc = bacc.Bacc(
            "TRN2", target_bir_lowering=False, debug=False, num_devices=NCORES)
        build_transformer(nc)
        nc.compile()
        _NC_CACHE["nc"] = nc
    return _NC_CACHE["nc"]


def prep_inputs(x, Wq, Wk, Wv, Wo, bo, W1, b1, W2, b2, g1, be1, g2, be2):
    """Host-side exact folding of LN affine params into weights/biases, plus
    layout packing. Returns (shared weight map, per-core xs shards)."""
    f32 = np.float32
    x = np.asarray(x, f32)
    Wq2 = np.asarray(Wq, f32).transpose(1, 0, 2).reshape(C, C)
    Wk2 = np.asarray(Wk, f32).transpose(1, 0, 2).reshape(C, C)
    Wv2 = np.asarray(Wv, f32).transpose(1, 0, 2).reshape(C, C)
    g1 = np.asarray(g1, f32)
    be1 = np.asarray(be1, f32)
    g2 = np.asarray(g2, f32)
    be2 = np.asarray(be2, f32)
    shared = {
        "wq": np.ascontiguousarray((g1[:, None] * Wq2).reshape(CK, P, C)),
        "wk": np.ascontiguousarray((g1[:, None] * Wk2).reshape(CK, P, C)),
        "wv": np.ascontiguousarray((g1[:, None] * Wv2).reshape(CK, P, C)),
        "wo": np.ascontiguousarray(np.asarray(Wo, f32).reshape(CK, P, C)),
        "w1": np.ascontiguousarray(
            (g2[:, None] * np.asarray(W1, f32)).reshape(CK, P, DFF)),
        "w2": np.ascontiguousarray(np.asarray(W2, f32).reshape(FK, P, C)),
        "bq": np.ascontiguousarray(be1 @ Wq2),
        "bk": np.ascontiguousarray(be1 @ Wk2),
        "bv": np.ascontiguousarray(be1 @ Wv2),
        "bo": np.ascontiguousarray(np.asarray(bo, f32)),
        "b1": np.ascontiguousarray(be2 @ np.asarray(W1, f32) + np.asarray(b1, f32)),
        "b2": np.ascontiguousarray(np.asarray(b2, f32)),
        "ident": np.eye(P, dtype=f32),
        "onesr": np.ones((1, P), dtype=f32),
    }
    shards = [
        np.ascontiguousarray(x[i * BL:(i + 1) * BL].reshape(NTOK, C))
        for i in range(NCORES)
    ]
    return shared, shards


def run_on_device(nc, shared, shards, trace=False, **kwargs):
    in_maps = [dict(shared, xs=shards[i]) for i in range(NCORES)]
    return run_bass_kernel_spmd(
        nc, in_maps, core_ids=list(range(NCORES)), trace=trace, **kwargs)


def kernel(**inputs):
    nc = get_nc()
    shared, shards = prep_inputs(**inputs)
    res = run_on_device(nc, shared, shards, trace=False)
    out = np.concatenate(
        [res.results[i]["out"].reshape(BL, T, C) for i in range(NCORES)], axis=0)
    return out.astype(np.float32)


# revision 11
# speedup vs baseline: 1.3866x; 1.3866x over previous
"""Trainium2 Bass kernel for a pre-norm transformer block (causal MHA + FFN).

Sharding: pure data-parallel over batch B=128 across 8 NeuronCores
(16 batches/core). No collectives.

v2 layout (per core, 4096 tokens processed as 8 supertiles of 512 tokens
= 2 batches each):
  - All TensorE operands are bf16 (fast weight load + 1 cycle/row at any
    free dim); PSUM accumulation and residual adds stay fp32.
  - LayerNorm token-major via bn_stats; rstd = Sqrt(reciprocal(var+eps))
    (DVE reciprocal + ACT Sqrt); normalized output written bf16 and
    PE-transposed to feature-major [c, t] supertile operands.
  - QKV projections at N=512; attention per batch: transposed scores
    [s, t] per head, exp on ScalarE -> bf16 probs, causal mask via
    gpsimd.affine_select, per-head row sums matmul'd into one [128, 6]
    psum column each -> single reciprocal, normalization applied as the
    per-partition ACT scale during the attn PSUM->SBUF copy.
  - Projection biases: per-partition ACT bias during PSUM->SBUF copies,
    or K=1 rank-1 matmul chunks for free-axis biases.
"""

import sys

for _p in ("/opt/trn_rl_repo",):
    if _p not in sys.path:
        sys.path.append(_p)

import numpy as np
import ml_dtypes

import concourse.bass as bass
import concourse.mybir as mybir
import concourse.tile as tile
from concourse import bacc
from concourse.bass_utils import run_bass_kernel_spmd

B, T, C, H, HS = 128, 256, 384, 6, 64
DFF = 4 * C
EPS = 1e-5
NCORES = 8
BL = B // NCORES          # batches per core (16)
NTOK = BL * T             # tokens per core (4096)
P = 128
CK = C // P               # channel chunks (3)
FK = DFF // P             # ffn chunks (12)
ST = 512                  # supertile tokens (2 batches)
NST = NTOK // ST          # supertiles per core (8)
NTT = ST // P             # token tiles per supertile (4)

F32 = mybir.dt.float32
BF16 = mybir.dt.bfloat16
AF = mybir.ActivationFunctionType
ALU = mybir.AluOpType
BF = ml_dtypes.bfloat16


def _layernorm_to_feature_major(nc, pools, x_tt, dst_T, identity, eps_tile):
    """LN token-major tiles of a supertile -> bf16 feature-major dst_T."""
    small, ps_tr, hn_pool = pools["small"], pools["ps_tr"], pools["hn"]
    for tt in range(NTT):
        xt = x_tt[:, tt, :]
        stats = small.tile([P, 6], F32, tag="stats")
        nc.vector.bn_stats(out=stats, in_=xt)
        mv = small.tile([P, 2], F32, tag="mv")
        nc.vector.bn_aggr(out=mv, in_=stats)
        # rstd = sqrt(1/(var+eps))
        iv = small.tile([P, 1], F32, tag="iv")
        nc.vector.tensor_scalar_add(out=iv, in0=mv[:, 1:2], scalar1=eps_tile)
        nc.vector.reciprocal(out=iv, in_=iv)
        rstd = small.tile([P, 1], F32, tag="rstd")
        nc.scalar.activation(out=rstd, in_=iv, func=AF.Sqrt)
        hn = hn_pool.tile([P, C], BF16, tag="hn")
        nc.vector.tensor_scalar(
            out=hn, in0=xt, scalar1=mv[:, 0:1], scalar2=rstd,
            op0=ALU.subtract, op1=ALU.mult,
        )
        for k in range(CK):
            ps = ps_tr.tile([P, P], BF16, tag="tr")
            nc.tensor.transpose(ps, hn[:, k * P:(k + 1) * P], identity)
            nc.vector.tensor_copy(out=dst_T[:, k, tt * P:(tt + 1) * P], in_=ps)


def build_transformer(nc):
    xs = nc.dram_tensor("xs", [NTOK, C], F32, kind="ExternalInput").ap()
    wq = nc.dram_tensor("wq", [CK, P, C], BF16, kind="ExternalInput").ap()
    wk = nc.dram_tensor("wk", [CK, P, C], BF16, kind="ExternalInput").ap()
    wv = nc.dram_tensor("wv", [CK, P, C], BF16, kind="ExternalInput").ap()
    wo = nc.dram_tensor("wo", [CK, P, C], BF16, kind="ExternalInput").ap()
    w1 = nc.dram_tensor("w1", [CK, P, DFF], BF16, kind="ExternalInput").ap()
    w2 = nc.dram_tensor("w2", [FK, P, C], BF16, kind="ExternalInput").ap()
    bq = nc.dram_tensor("bq", [C], F32, kind="ExternalInput").ap()
    bk = nc.dram_tensor("bk", [C], F32, kind="ExternalInput").ap()
    bv = nc.dram_tensor("bv", [C], BF16, kind="ExternalInput").ap()
    bo = nc.dram_tensor("bo", [C], BF16, kind="ExternalInput").ap()
    b1 = nc.dram_tensor("b1", [DFF], F32, kind="ExternalInput").ap()
    b2 = nc.dram_tensor("b2", [C], BF16, kind="ExternalInput").ap()
    ident = nc.dram_tensor("ident", [P, P], BF16, kind="ExternalInput").ap()
    onesr = nc.dram_tensor("onesr", [1, P], BF16, kind="ExternalInput").ap()
    out = nc.dram_tensor("out", [NTOK, C], F32, kind="ExternalOutput").ap()

    from contextlib import ExitStack
    with tile.TileContext(nc) as tc, ExitStack() as ctx:
        const = ctx.enter_context(tc.tile_pool(name="const", bufs=1))
        io_pool = ctx.enter_context(tc.tile_pool(name="io", bufs=2))
        act_pool = ctx.enter_context(tc.tile_pool(name="act", bufs=2))
        hn_pool = ctx.enter_context(tc.tile_pool(name="hn", bufs=3))
        wei_pool = ctx.enter_context(tc.tile_pool(name="wei", bufs=4))
        small = ctx.enter_context(tc.tile_pool(name="small", bufs=6))
        rc_pool = ctx.enter_context(tc.tile_pool(name="rc", bufs=6))
        ps_tr = ctx.enter_context(tc.tile_pool(name="ps_tr", bufs=1, space="PSUM"))
        ps_mm = ctx.enter_context(tc.tile_pool(name="ps_mm", bufs=2, space="PSUM"))
        ps_big = ctx.enter_context(tc.tile_pool(name="ps_big", bufs=1, space="PSUM"))
        ps_attn = ctx.enter_context(tc.tile_pool(name="ps_attn", bufs=2, space="PSUM"))
        ps_sum = ctx.enter_context(tc.tile_pool(name="ps_sum", bufs=2, space="PSUM"))
        pools = {"small": small, "hn": hn_pool, "ps_tr": ps_tr}

        # ---- persistent constants ----
        wq_sb = const.tile([P, CK, C], BF16)
        nc.sync.dma_start(out=wq_sb, in_=wq.rearrange("k p d -> p k d"))
        wk_sb = const.tile([P, CK, C], BF16)
        nc.sync.dma_start(out=wk_sb, in_=wk.rearrange("k p d -> p k d"))
        wv_sb = const.tile([P, CK, C], BF16)
        nc.sync.dma_start(out=wv_sb, in_=wv.rearrange("k p d -> p k d"))
        wo_sb = const.tile([P, CK, C], BF16)
        nc.sync.dma_start(out=wo_sb, in_=wo.rearrange("k p d -> p k d"))
        w1_sb = const.tile([P, CK, DFF], BF16)
        nc.sync.dma_start(out=w1_sb, in_=w1.rearrange("k p d -> p k d"))
        w2_sb = const.tile([P, FK, C], BF16)
        nc.sync.dma_start(out=w2_sb, in_=w2.rearrange("k p d -> p k d"))
        bq_sb = const.tile([P, CK], F32)
        nc.sync.dma_start(out=bq_sb, in_=bq.rearrange("(k p) -> p k", p=P))
        bk_sb = const.tile([P, CK], F32)
        nc.sync.dma_start(out=bk_sb, in_=bk.rearrange("(k p) -> p k", p=P))
        b1_sb = const.tile([P, FK], F32)
        nc.sync.dma_start(out=b1_sb, in_=b1.rearrange("(f p) -> p f", p=P))
        bv_row = const.tile([1, C], BF16)
        nc.sync.dma_start(out=bv_row, in_=bv.rearrange("(a d) -> a d", a=1))
        bo_row = const.tile([1, C], BF16)
        nc.sync.dma_start(out=bo_row, in_=bo.rearrange("(a d) -> a d", a=1))
        b2_row = const.tile([1, C], BF16)
        nc.sync.dma_start(out=b2_row, in_=b2.rearrange("(a d) -> a d", a=1))
        ones1 = const.tile([1, P], BF16)
        nc.sync.dma_start(out=ones1, in_=onesr)
        ones_bf = const.tile([P, 1], BF16)
        nc.vector.memset(ones_bf, 1.0)
        identity = const.tile([P, P], BF16)
        nc.sync.dma_start(out=identity, in_=ident)
        eps_tile = const.tile([P, 1], F32)
        nc.vector.memset(eps_tile, EPS)

        for g in range(NST):
            t0 = g * ST
            # ---- load x supertile (4 token tiles) ----
            xa = io_pool.tile([P, NTT, C], F32, tag="xa")
            nc.sync.dma_start(
                out=xa, in_=xs[t0:t0 + ST, :].rearrange("(tt p) c -> p tt c", p=P))

            # ---- LN1 -> h1T bf16 feature-major [c, 512] ----
            h1T = act_pool.tile([P, CK, ST], BF16, tag="h1T")
            _layernorm_to_feature_major(nc, pools, xa, h1T, identity, eps_tile)

            # ---- QKV projections (N=512) ----
            QT = act_pool.tile([P, CK, ST], BF16, tag="QT")
            KT = act_pool.tile([P, CK, ST], BF16, tag="KT")
            for m in range(CK):
                psq = ps_mm.tile([P, ST], F32, tag="mm")
                for k in range(CK):
                    nc.tensor.matmul(
                        psq, wq_sb[:, k, m * P:(m + 1) * P], h1T[:, k, :],
                        start=(k == 0), stop=(k == CK - 1))
                nc.scalar.activation(
                    out=QT[:, m, :], in_=psq, func=AF.Identity,
                    bias=bq_sb[:, m:m + 1])
                psk = ps_mm.tile([P, ST], F32, tag="mm")
                for k in range(CK):
                    nc.tensor.matmul(
                        psk, wk_sb[:, k, m * P:(m + 1) * P], h1T[:, k, :],
                        start=(k == 0), stop=(k == CK - 1))
                nc.scalar.activation(
                    out=KT[:, m, :], in_=psk, func=AF.Identity,
                    bias=bk_sb[:, m:m + 1])
            Vtm = act_pool.tile([P, NTT, C], BF16, tag="Vtm")
            for tt in range(NTT):
                psv = ps_big.tile([P, C], F32, tag="big")
                for k in range(CK):
                    nc.tensor.matmul(
                        psv, h1T[:, k, tt * P:(tt + 1) * P], wv_sb[:, k, :],
                        start=(k == 0), stop=False)
                nc.tensor.matmul(psv, ones1, bv_row, start=False, stop=True)
                nc.vector.tensor_copy(out=Vtm[:, tt, :], in_=psv)

            # ---- attention per batch ----
            attn_sb = hn_pool.tile([P, NTT, C], BF16, tag="attn_sb")
            for b2 in range(2):
                co = b2 * T          # column offset of this batch in supertile
                vo = b2 * 2          # Vtm token-tile offset
                attn_ps = [
                    ps_attn.tile([P, C], F32, tag="attn", name=f"aps{g}_{b2}_{tt}")
                    for tt in range(2)
                ]
                psr6 = [
                    ps_sum.tile([P, H], F32, tag="sum", name=f"psr{g}_{b2}_{tt}")
                    for tt in range(2)
                ]
                for h in range(H):
                    po = 64 * (h % 2)
                    m = h // 2
                    qh = QT[po:po + HS, m, co:co + T]
                    kh = KT[po:po + HS, m, co:co + T]
                    weiT = wei_pool.tile([P, 2, T], BF16, tag="weiT")
                    for sc in range(2):
                        pss = ps_mm.tile([P, T], F32, tag="mm")
                        nc.tensor.matmul(
                            pss, kh[:, sc * P:(sc + 1) * P], qh,
                            start=True, stop=True)
                        if sc == 0:
                            nc.scalar.activation(
                                out=weiT[:, 0, :], in_=pss, func=AF.Exp,
                                scale=HS ** -0.5)
                        else:
                            nc.scalar.activation(
                                out=weiT[:, 1, P:], in_=pss[:, P:], func=AF.Exp,
                                scale=HS ** -0.5)
                    # causal mask on diagonal blocks: keep where t >= s
                    nc.gpsimd.affine_select(
                        out=weiT[:, 0, 0:P], in_=weiT[:, 0, 0:P],
                        compare_op=ALU.is_ge, fill=0.0, base=0,
                        pattern=[[1, P]], channel_multiplier=-1)
                    nc.gpsimd.affine_select(
                        out=weiT[:, 1, P:], in_=weiT[:, 1, P:],
                        compare_op=ALU.is_ge, fill=0.0, base=0,
                        pattern=[[1, P]], channel_multiplier=-1)
                    # row sums -> column h of shared [128, 6] psum
                    nc.tensor.matmul(psr6[0][:, h:h + 1], weiT[:, 0, 0:P],
                                     ones_bf, start=True, stop=True)
                    nc.tensor.matmul(psr6[1][:, h:h + 1], weiT[:, 0, P:],
                                     ones_bf, start=True, stop=False)
                    nc.tensor.matmul(psr6[1][:, h:h + 1], weiT[:, 1, P:],
                                     ones_bf, start=False, stop=True)
                    # attnV token-major into per-head column slice
                    for tt in range(2):
                        dst = attn_ps[tt][:, h * HS:(h + 1) * HS]
                        if tt == 0:
                            nc.tensor.matmul(
                                dst, weiT[:, 0, 0:P],
                                Vtm[:, vo, h * HS:(h + 1) * HS],
                                start=True, stop=True)
                        else:
                            nc.tensor.matmul(
                                dst, weiT[:, 0, P:],
                                Vtm[:, vo, h * HS:(h + 1) * HS],
                                start=True, stop=False)
                            nc.tensor.matmul(
                                dst, weiT[:, 1, P:],
                                Vtm[:, vo + 1, h * HS:(h + 1) * HS],
                                start=False, stop=True)
                # one reciprocal for all 6 head sums; normalize via the
                # per-partition ACT scale during the PSUM->SBUF copy
                for tt in range(2):
                    rc6 = rc_pool.tile([P, H], F32, tag="rc")
                    nc.vector.reciprocal(out=rc6, in_=psr6[tt])
                    for h in range(H):
                        nc.scalar.activation(
                            out=attn_sb[:, vo + tt, h * HS:(h + 1) * HS],
                            in_=attn_ps[tt][:, h * HS:(h + 1) * HS],
                            func=AF.Identity, scale=rc6[:, h:h + 1])

            # ---- transpose attn to feature-major; Wo + residual ----
            attn_T = act_pool.tile([P, CK, ST], BF16, tag="attnT")
            for tt in range(NTT):
                for k in range(CK):
                    pst = ps_tr.tile([P, P], BF16, tag="tr")
                    nc.tensor.transpose(
                        pst, attn_sb[:, tt, k * P:(k + 1) * P], identity)
                    nc.vector.tensor_copy(
                        out=attn_T[:, k, tt * P:(tt + 1) * P], in_=pst)
            xmid = io_pool.tile([P, NTT, C], F32, tag="xmid")
            for tt in range(NTT):
                pso = ps_big.tile([P, C], F32, tag="big")
                for k in range(CK):
                    nc.tensor.matmul(
                        pso, attn_T[:, k, tt * P:(tt + 1) * P], wo_sb[:, k, :],
                        start=(k == 0), stop=False)
                nc.tensor.matmul(pso, ones1, bo_row, start=False, stop=True)
                nc.vector.tensor_add(out=xmid[:, tt, :], in0=xa[:, tt, :], in1=pso)

            # ---- LN2 -> h2T; FFN ----
            h2T = act_pool.tile([P, CK, ST], BF16, tag="h2T")
            _layernorm_to_feature_major(nc, pools, xmid, h2T, identity, eps_tile)

            ff1T = act_pool.tile([P, FK, ST], BF16, tag="ff1T")
            for f in range(FK):
                psf = ps_mm.tile([P, ST], F32, tag="mm")
                for k in range(CK):
                    nc.tensor.matmul(
                        psf, w1_sb[:, k, f * P:(f + 1) * P], h2T[:, k, :],
                        start=(k == 0), stop=(k == CK - 1))
                nc.scalar.activation(
                    out=ff1T[:, f, :], in_=psf, func=AF.Relu, bias=b1_sb[:, f:f + 1])
            for tt in range(NTT):
                ps2 = ps_big.tile([P, C], F32, tag="big")
                for f in range(FK):
                    nc.tensor.matmul(
                        ps2, ff1T[:, f, tt * P:(tt + 1) * P], w2_sb[:, f, :],
                        start=(f == 0), stop=False)
                nc.tensor.matmul(ps2, ones1, b2_row, start=False, stop=True)
                yt = io_pool.tile([P, C], F32, tag="yt")
                nc.vector.tensor_add(out=yt, in0=xmid[:, tt, :], in1=ps2)
                nc.sync.dma_start(
                    out=out[t0 + tt * P: t0 + (tt + 1) * P, :], in_=yt)
    return nc


_NC_CACHE = {}


def get_nc():
    if "nc" not in _NC_CACHE:
        nc = bacc.Bacc(
            "TRN2", target_bir_lowering=False, debug=False, num_devices=NCORES)
        build_transformer(nc)
        nc.compile()
        _NC_CACHE["nc"] = nc
    return _NC_CACHE["nc"]


def prep_inputs(x, Wq, Wk, Wv, Wo, bo, W1, b1, W2, b2, g1, be1, g2, be2):
    """Host-side exact folding of LN affine params into weights/biases, plus
    layout packing and bf16 casts."""
    f32 = np.float32
    x = np.asarray(x, f32)
    Wq2 = np.asarray(Wq, f32).transpose(1, 0, 2).reshape(C, C)
    Wk2 = np.asarray(Wk, f32).transpose(1, 0, 2).reshape(C, C)
    Wv2 = np.asarray(Wv, f32).transpose(1, 0, 2).reshape(C, C)
    g1 = np.asarray(g1, f32)
    be1 = np.asarray(be1, f32)
    g2 = np.asarray(g2, f32)
    be2 = np.asarray(be2, f32)
    bf = lambda a: np.ascontiguousarray(np.asarray(a, f32)).astype(BF)
    shared = {
        "wq": bf((g1[:, None] * Wq2).reshape(CK, P, C)),
        "wk": bf((g1[:, None] * Wk2).reshape(CK, P, C)),
        "wv": bf((g1[:, None] * Wv2).reshape(CK, P, C)),
        "wo": bf(np.asarray(Wo, f32).reshape(CK, P, C)),
        "w1": bf((g2[:, None] * np.asarray(W1, f32)).reshape(CK, P, DFF)),
        "w2": bf(np.asarray(W2, f32).reshape(FK, P, C)),
        "bq": np.ascontiguousarray(be1 @ Wq2),
        "bk": np.ascontiguousarray(be1 @ Wk2),
        "bv": bf(be1 @ Wv2),
        "bo": bf(np.asarray(bo, f32)),
        "b1": np.ascontiguousarray(be2 @ np.asarray(W1, f32) + np.asarray(b1, f32)),
        "b2": bf(np.asarray(b2, f32)),
        "ident": np.eye(P, dtype=f32).astype(BF),
        "onesr": np.ones((1, P), dtype=f32).astype(BF),
    }
    shards = [
        np.ascontiguousarray(x[i * BL:(i + 1) * BL].reshape(NTOK, C))
        for i in range(NCORES)
    ]
    return shared, shards


def run_on_device(nc, shared, shards, trace=False, **kwargs):
    in_maps = [dict(shared, xs=shards[i]) for i in range(NCORES)]
    return run_bass_kernel_spmd(
        nc, in_maps, core_ids=list(range(NCORES)), trace=trace, **kwargs)


def kernel(**inputs):
    nc = get_nc()
    shared, shards = prep_inputs(**inputs)
    res = run_on_device(nc, shared, shards, trace=False)
    out = np.concatenate(
        [res.results[i]["out"].reshape(BL, T, C) for i in range(NCORES)], axis=0)
    return out.astype(np.float32)


# revision 13
# speedup vs baseline: 1.6521x; 1.1915x over previous
"""Trainium2 Bass kernel for a pre-norm transformer block (causal MHA + FFN).

Sharding: pure data-parallel over batch B=128 across 8 NeuronCores
(16 batches/core). No collectives.

v2 layout (per core, 4096 tokens processed as 8 supertiles of 512 tokens
= 2 batches each):
  - All TensorE operands are bf16 (fast weight load + 1 cycle/row at any
    free dim); PSUM accumulation and residual adds stay fp32.
  - LayerNorm token-major via bn_stats; rstd = Sqrt(reciprocal(var+eps))
    (DVE reciprocal + ACT Sqrt); normalized output written bf16 and
    PE-transposed to feature-major [c, t] supertile operands.
  - QKV projections at N=512; attention per batch: transposed scores
    [s, t] per head, exp on ScalarE -> bf16 probs, causal mask via
    gpsimd.affine_select, per-head row sums matmul'd into one [128, 6]
    psum column each -> single reciprocal, normalization applied as the
    per-partition ACT scale during the attn PSUM->SBUF copy.
  - Projection biases: per-partition ACT bias during PSUM->SBUF copies,
    or K=1 rank-1 matmul chunks for free-axis biases.
"""

import sys

for _p in ("/opt/trn_rl_repo",):
    if _p not in sys.path:
        sys.path.append(_p)

import numpy as np
import ml_dtypes

import concourse.bass as bass
import concourse.mybir as mybir
import concourse.tile as tile
from concourse import bacc
from concourse.bass_utils import run_bass_kernel_spmd

B, T, C, H, HS = 128, 256, 384, 6, 64
DFF = 4 * C
EPS = 1e-5
NCORES = 8
BL = B // NCORES          # batches per core (16)
NTOK = BL * T             # tokens per core (4096)
P = 128
CK = C // P               # channel chunks (3)
FK = DFF // P             # ffn chunks (12)
ST = 512                  # supertile tokens (2 batches)
NST = NTOK // ST          # supertiles per core (8)
NTT = ST // P             # token tiles per supertile (4)

F32 = mybir.dt.float32
BF16 = mybir.dt.bfloat16
AF = mybir.ActivationFunctionType
ALU = mybir.AluOpType
BF = ml_dtypes.bfloat16


def _layernorm_to_feature_major(nc, pools, x_tt, dst_T, identity, eps_tile):
    """LN token-major tiles of a supertile -> bf16 feature-major dst_T."""
    small, ps_tr, hn_pool = pools["small"], pools["ps_tr"], pools["hn"]
    for tt in range(NTT):
        xt = x_tt[:, tt, :]
        stats = small.tile([P, 6], F32, tag="stats")
        nc.vector.bn_stats(out=stats, in_=xt)
        mv = small.tile([P, 2], F32, tag="mv")
        nc.vector.bn_aggr(out=mv, in_=stats)
        # rstd = sqrt(1/(var+eps))
        iv = small.tile([P, 1], F32, tag="iv")
        nc.vector.tensor_scalar_add(out=iv, in0=mv[:, 1:2], scalar1=eps_tile)
        nc.vector.reciprocal(out=iv, in_=iv)
        rstd = small.tile([P, 1], F32, tag="rstd")
        nc.scalar.activation(out=rstd, in_=iv, func=AF.Sqrt)
        hn = hn_pool.tile([P, C], BF16, tag="hn")
        nc.vector.tensor_scalar(
            out=hn, in0=xt, scalar1=mv[:, 0:1], scalar2=rstd,
            op0=ALU.subtract, op1=ALU.mult,
        )
        for k in range(CK):
            ps = ps_tr.tile([P, P], BF16, tag="tr")
            nc.tensor.transpose(ps, hn[:, k * P:(k + 1) * P], identity)
            nc.vector.tensor_copy(out=dst_T[:, k, tt * P:(tt + 1) * P], in_=ps)


def build_transformer(nc):
    xs = nc.dram_tensor("xs", [NTOK, C], F32, kind="ExternalInput").ap()
    wq = nc.dram_tensor("wq", [CK, P, C], BF16, kind="ExternalInput").ap()
    wk = nc.dram_tensor("wk", [CK, P, C], BF16, kind="ExternalInput").ap()
    wv = nc.dram_tensor("wv", [CK, P, C], BF16, kind="ExternalInput").ap()
    wo = nc.dram_tensor("wo", [CK, P, C], BF16, kind="ExternalInput").ap()
    w1 = nc.dram_tensor("w1", [CK, P, DFF], BF16, kind="ExternalInput").ap()
    w2 = nc.dram_tensor("w2", [FK, P, C], BF16, kind="ExternalInput").ap()
    bq = nc.dram_tensor("bq", [C], F32, kind="ExternalInput").ap()
    bk = nc.dram_tensor("bk", [C], F32, kind="ExternalInput").ap()
    bv = nc.dram_tensor("bv", [C], BF16, kind="ExternalInput").ap()
    bo = nc.dram_tensor("bo", [C], BF16, kind="ExternalInput").ap()
    b1 = nc.dram_tensor("b1", [DFF], F32, kind="ExternalInput").ap()
    b2 = nc.dram_tensor("b2", [C], BF16, kind="ExternalInput").ap()
    ident = nc.dram_tensor("ident", [P, P], BF16, kind="ExternalInput").ap()
    onesr = nc.dram_tensor("onesr", [1, P], BF16, kind="ExternalInput").ap()
    out = nc.dram_tensor("out", [NTOK, C], F32, kind="ExternalOutput").ap()

    from contextlib import ExitStack
    with tile.TileContext(nc) as tc, ExitStack() as ctx:
        const = ctx.enter_context(tc.tile_pool(name="const", bufs=1))
        io_pool = ctx.enter_context(tc.tile_pool(name="io", bufs=2))
        act_pool = ctx.enter_context(tc.tile_pool(name="act", bufs=2))
        hn_pool = ctx.enter_context(tc.tile_pool(name="hn", bufs=3))
        wei_pool = ctx.enter_context(tc.tile_pool(name="wei", bufs=4))
        small = ctx.enter_context(tc.tile_pool(name="small", bufs=6))
        rc_pool = ctx.enter_context(tc.tile_pool(name="rc", bufs=6))
        ps_tr = ctx.enter_context(tc.tile_pool(name="ps_tr", bufs=3, space="PSUM"))
        ps_mm = ctx.enter_context(tc.tile_pool(name="ps_mm", bufs=2, space="PSUM"))
        ps_big = ctx.enter_context(tc.tile_pool(name="ps_big", bufs=1, space="PSUM"))
        ps_attn = ctx.enter_context(tc.tile_pool(name="ps_attn", bufs=2, space="PSUM"))
        pools = {"small": small, "hn": hn_pool, "ps_tr": ps_tr}

        # ---- persistent constants ----
        wq_sb = const.tile([P, CK, C], BF16)
        nc.sync.dma_start(out=wq_sb, in_=wq.rearrange("k p d -> p k d"))
        wk_sb = const.tile([P, CK, C], BF16)
        nc.sync.dma_start(out=wk_sb, in_=wk.rearrange("k p d -> p k d"))
        wv_sb = const.tile([P, CK, C], BF16)
        nc.sync.dma_start(out=wv_sb, in_=wv.rearrange("k p d -> p k d"))
        wo_sb = const.tile([P, CK, C], BF16)
        nc.sync.dma_start(out=wo_sb, in_=wo.rearrange("k p d -> p k d"))
        w1_sb = const.tile([P, CK, DFF], BF16)
        nc.sync.dma_start(out=w1_sb, in_=w1.rearrange("k p d -> p k d"))
        w2_sb = const.tile([P, FK, C], BF16)
        nc.sync.dma_start(out=w2_sb, in_=w2.rearrange("k p d -> p k d"))
        bq_sb = const.tile([P, CK], F32)
        nc.sync.dma_start(out=bq_sb, in_=bq.rearrange("(k p) -> p k", p=P))
        bk_sb = const.tile([P, CK], F32)
        nc.sync.dma_start(out=bk_sb, in_=bk.rearrange("(k p) -> p k", p=P))
        b1_sb = const.tile([P, FK], F32)
        nc.sync.dma_start(out=b1_sb, in_=b1.rearrange("(f p) -> p f", p=P))
        bv_row = const.tile([1, C], BF16)
        nc.sync.dma_start(out=bv_row, in_=bv.rearrange("(a d) -> a d", a=1))
        bo_row = const.tile([1, C], BF16)
        nc.sync.dma_start(out=bo_row, in_=bo.rearrange("(a d) -> a d", a=1))
        b2_row = const.tile([1, C], BF16)
        nc.sync.dma_start(out=b2_row, in_=b2.rearrange("(a d) -> a d", a=1))
        ones1 = const.tile([1, P], BF16)
        nc.sync.dma_start(out=ones1, in_=onesr)
        ones_bf = const.tile([P, 1], BF16)
        nc.vector.memset(ones_bf, 1.0)
        identity = const.tile([P, P], BF16)
        nc.sync.dma_start(out=identity, in_=ident)
        eps_tile = const.tile([P, 1], F32)
        nc.vector.memset(eps_tile, EPS)

        def load_and_ln1(g):
            """DMA x supertile g and LayerNorm it into a fresh h1T."""
            t0 = g * ST
            xa = io_pool.tile([P, NTT, C], F32, tag="xa", name=f"xa{g}")
            nc.sync.dma_start(
                out=xa, in_=xs[t0:t0 + ST, :].rearrange("(tt p) c -> p tt c", p=P))
            h1T = act_pool.tile([P, CK, ST], BF16, tag="h1T", name=f"h1T{g}")
            _layernorm_to_feature_major(nc, pools, xa, h1T, identity, eps_tile)
            return xa, h1T

        nxt = load_and_ln1(0)
        for g in range(NST):
            t0 = g * ST
            xa, h1T = nxt

            # ---- QKV projections (N=512) ----
            QT = act_pool.tile([P, CK, ST], BF16, tag="QT")
            KT = act_pool.tile([P, CK, ST], BF16, tag="KT")
            for m in range(CK):
                psq = ps_mm.tile([P, ST], F32, tag="mm")
                for k in range(CK):
                    nc.tensor.matmul(
                        psq, wq_sb[:, k, m * P:(m + 1) * P], h1T[:, k, :],
                        start=(k == 0), stop=(k == CK - 1))
                nc.vector.tensor_scalar_add(
                    out=QT[:, m, :], in0=psq, scalar1=bq_sb[:, m:m + 1])
                psk = ps_mm.tile([P, ST], F32, tag="mm")
                for k in range(CK):
                    nc.tensor.matmul(
                        psk, wk_sb[:, k, m * P:(m + 1) * P], h1T[:, k, :],
                        start=(k == 0), stop=(k == CK - 1))
                nc.vector.tensor_scalar_add(
                    out=KT[:, m, :], in0=psk, scalar1=bk_sb[:, m:m + 1])
            # V token-major, augmented with a ones column per head (65-wide
            # head slices) so attnV matmuls also produce the softmax row sums
            Vtm = act_pool.tile([P, NTT, H * 65], BF16, tag="Vtm")
            for tt in range(NTT):
                psv = ps_big.tile([P, C], F32, tag="big")
                for k in range(CK):
                    nc.tensor.matmul(
                        psv, h1T[:, k, tt * P:(tt + 1) * P], wv_sb[:, k, :],
                        start=(k == 0), stop=False)
                nc.tensor.matmul(psv, ones1, bv_row, start=False, stop=True)
                vview = Vtm[:, tt, :].rearrange("p (h e) -> p h e", e=65)
                nc.vector.tensor_copy(
                    out=vview[:, :, 0:HS],
                    in_=psv.rearrange("p (h e) -> p h e", e=HS))
                nc.gpsimd.memset(vview[:, :, HS:65], 1.0)

            # ---- attention per batch ----
            attn_sb = hn_pool.tile([P, NTT, C], BF16, tag="attn_sb")
            for b2 in range(2):
                co = b2 * T          # column offset of this batch in supertile
                vo = b2 * 2          # Vtm token-tile offset
                attn_ps = [
                    ps_attn.tile([P, H * 65], F32, tag="attn",
                                 name=f"aps{g}_{b2}_{tt}")
                    for tt in range(2)
                ]
                for h in range(H):
                    po = 64 * (h % 2)
                    m = h // 2
                    qh = QT[po:po + HS, m, co:co + T]
                    kh = KT[po:po + HS, m, co:co + T]
                    weiT = wei_pool.tile([P, 2, T], BF16, tag="weiT")
                    pss = ps_mm.tile([P, ST], F32, tag="mm")
                    nc.tensor.matmul(
                        pss[:, 0:T], kh[:, 0:P], qh, start=True, stop=True)
                    nc.scalar.activation(
                        out=weiT[:, 0, :], in_=pss[:, 0:T], func=AF.Exp,
                        scale=HS ** -0.5)
                    nc.tensor.matmul(
                        pss[:, T:T + P], kh[:, P:], qh[:, P:],
                        start=True, stop=True)
                    nc.scalar.activation(
                        out=weiT[:, 1, P:], in_=pss[:, T:T + P], func=AF.Exp,
                        scale=HS ** -0.5)
                    # causal mask on diagonal blocks: keep where t >= s
                    nc.gpsimd.affine_select(
                        out=weiT[:, 0, 0:P], in_=weiT[:, 0, 0:P],
                        compare_op=ALU.is_ge, fill=0.0, base=0,
                        pattern=[[1, P]], channel_multiplier=-1)
                    nc.gpsimd.affine_select(
                        out=weiT[:, 1, P:], in_=weiT[:, 1, P:],
                        compare_op=ALU.is_ge, fill=0.0, base=0,
                        pattern=[[1, P]], channel_multiplier=-1)
                    # attnV + row sums in one matmul per (s-chunk, t-tile):
                    # rhs is the 65-wide augmented V head slice
                    for tt in range(2):
                        dst = attn_ps[tt][:, h * 65:(h + 1) * 65]
                        if tt == 0:
                            nc.tensor.matmul(
                                dst, weiT[:, 0, 0:P],
                                Vtm[:, vo, h * 65:(h + 1) * 65],
                                start=True, stop=True)
                        else:
                            nc.tensor.matmul(
                                dst, weiT[:, 0, P:],
                                Vtm[:, vo, h * 65:(h + 1) * 65],
                                start=True, stop=False)
                            nc.tensor.matmul(
                                dst, weiT[:, 1, P:],
                                Vtm[:, vo + 1, h * 65:(h + 1) * 65],
                                start=False, stop=True)
                # one reciprocal over the 6 interleaved row sums, then
                # normalize via the per-partition ACT scale during the copy
                for tt in range(2):
                    aview = attn_ps[tt].rearrange("p (h e) -> p h e", e=65)
                    rc6 = rc_pool.tile([P, H], F32, tag="rc")
                    nc.vector.reciprocal(out=rc6, in_=aview[:, :, HS])
                    for h in range(H):
                        nc.scalar.activation(
                            out=attn_sb[:, vo + tt, h * HS:(h + 1) * HS],
                            in_=aview[:, h, 0:HS],
                            func=AF.Identity, scale=rc6[:, h:h + 1])

            # ---- transpose attn to feature-major; Wo + residual ----
            attn_T = act_pool.tile([P, CK, ST], BF16, tag="attnT")
            for tt in range(NTT):
                for k in range(CK):
                    pst = ps_tr.tile([P, P], BF16, tag="tr")
                    nc.tensor.transpose(
                        pst, attn_sb[:, tt, k * P:(k + 1) * P], identity)
                    nc.vector.tensor_copy(
                        out=attn_T[:, k, tt * P:(tt + 1) * P], in_=pst)
            xmid = io_pool.tile([P, NTT, C], F32, tag="xmid")
            for tt in range(NTT):
                pso = ps_big.tile([P, C], F32, tag="big")
                for k in range(CK):
                    nc.tensor.matmul(
                        pso, attn_T[:, k, tt * P:(tt + 1) * P], wo_sb[:, k, :],
                        start=(k == 0), stop=False)
                nc.tensor.matmul(pso, ones1, bo_row, start=False, stop=True)
                nc.vector.tensor_add(out=xmid[:, tt, :], in0=xa[:, tt, :], in1=pso)

            # ---- LN2 -> h2T ----
            h2T = act_pool.tile([P, CK, ST], BF16, tag="h2T")
            _layernorm_to_feature_major(nc, pools, xmid, h2T, identity, eps_tile)

            # hoist next supertile's load + LN1 here so its Sqrt ops sit
            # adjacent to LN2's (one ACT table load covers both)
            if g + 1 < NST:
                nxt = load_and_ln1(g + 1)

            # ---- FFN ----
            ff1T = act_pool.tile([P, FK, ST], BF16, tag="ff1T")
            for f in range(FK):
                psf = ps_mm.tile([P, ST], F32, tag="mm")
                for k in range(CK):
                    nc.tensor.matmul(
                        psf, w1_sb[:, k, f * P:(f + 1) * P], h2T[:, k, :],
                        start=(k == 0), stop=(k == CK - 1))
                nc.scalar.activation(
                    out=ff1T[:, f, :], in_=psf, func=AF.Relu, bias=b1_sb[:, f:f + 1])
            for tt in range(NTT):
                ps2 = ps_big.tile([P, C], F32, tag="big")
                for f in range(FK):
                    nc.tensor.matmul(
                        ps2, ff1T[:, f, tt * P:(tt + 1) * P], w2_sb[:, f, :],
                        start=(f == 0), stop=False)
                nc.tensor.matmul(ps2, ones1, b2_row, start=False, stop=True)
                yt = io_pool.tile([P, C], F32, tag="yt")
                nc.vector.tensor_add(out=yt, in0=xmid[:, tt, :], in1=ps2)
                nc.sync.dma_start(
                    out=out[t0 + tt * P: t0 + (tt + 1) * P, :], in_=yt)
    return nc


_NC_CACHE = {}


def get_nc():
    if "nc" not in _NC_CACHE:
        nc = bacc.Bacc(
            "TRN2", target_bir_lowering=False, debug=False, num_devices=NCORES)
        build_transformer(nc)
        nc.compile()
        _NC_CACHE["nc"] = nc
    return _NC_CACHE["nc"]


def prep_inputs(x, Wq, Wk, Wv, Wo, bo, W1, b1, W2, b2, g1, be1, g2, be2):
    """Host-side exact folding of LN affine params into weights/biases, plus
    layout packing and bf16 casts."""
    f32 = np.float32
    x = np.asarray(x, f32)
    Wq2 = np.asarray(Wq, f32).transpose(1, 0, 2).reshape(C, C)
    Wk2 = np.asarray(Wk, f32).transpose(1, 0, 2).reshape(C, C)
    Wv2 = np.asarray(Wv, f32).transpose(1, 0, 2).reshape(C, C)
    g1 = np.asarray(g1, f32)
    be1 = np.asarray(be1, f32)
    g2 = np.asarray(g2, f32)
    be2 = np.asarray(be2, f32)
    bf = lambda a: np.ascontiguousarray(np.asarray(a, f32)).astype(BF)
    shared = {
        "wq": bf((g1[:, None] * Wq2).reshape(CK, P, C)),
        "wk": bf((g1[:, None] * Wk2).reshape(CK, P, C)),
        "wv": bf((g1[:, None] * Wv2).reshape(CK, P, C)),
        "wo": bf(np.asarray(Wo, f32).reshape(CK, P, C)),
        "w1": bf((g2[:, None] * np.asarray(W1, f32)).reshape(CK, P, DFF)),
        "w2": bf(np.asarray(W2, f32).reshape(FK, P, C)),
        "bq": np.ascontiguousarray(be1 @ Wq2),
        "bk": np.ascontiguousarray(be1 @ Wk2),
        "bv": bf(be1 @ Wv2),
        "bo": bf(np.asarray(bo, f32)),
        "b1": np.ascontiguousarray(be2 @ np.asarray(W1, f32) + np.asarray(b1, f32)),
        "b2": bf(np.asarray(b2, f32)),
        "ident": np.eye(P, dtype=f32).astype(BF),
        "onesr": np.ones((1, P), dtype=f32).astype(BF),
    }
    shards = [
        np.ascontiguousarray(x[i * BL:(i + 1) * BL].reshape(NTOK, C))
        for i in range(NCORES)
    ]
    return shared, shards


def run_on_device(nc, shared, shards, trace=False, **kwargs):
    in_maps = [dict(shared, xs=shards[i]) for i in range(NCORES)]
    return run_bass_kernel_spmd(
        nc, in_maps, core_ids=list(range(NCORES)), trace=trace, **kwargs)


def kernel(**inputs):
    nc = get_nc()
    shared, shards = prep_inputs(**inputs)
    res = run_on_device(nc, shared, shards, trace=False)
    out = np.concatenate(
        [res.results[i]["out"].reshape(BL, T, C) for i in range(NCORES)], axis=0)
    return out.astype(np.float32)


# revision 14
# speedup vs baseline: 1.7581x; 1.0641x over previous
"""Trainium2 Bass kernel for a pre-norm transformer block (causal MHA + FFN).

Sharding: pure data-parallel over batch B=128 across 8 NeuronCores
(16 batches/core). No collectives.

v2 layout (per core, 4096 tokens processed as 8 supertiles of 512 tokens
= 2 batches each):
  - All TensorE operands are bf16 (fast weight load + 1 cycle/row at any
    free dim); PSUM accumulation and residual adds stay fp32.
  - LayerNorm token-major via bn_stats; rstd = Sqrt(reciprocal(var+eps))
    (DVE reciprocal + ACT Sqrt); normalized output written bf16 and
    PE-transposed to feature-major [c, t] supertile operands.
  - QKV projections at N=512; attention per batch: transposed scores
    [s, t] per head, exp on ScalarE -> bf16 probs, causal mask via
    gpsimd.affine_select, per-head row sums matmul'd into one [128, 6]
    psum column each -> single reciprocal, normalization applied as the
    per-partition ACT scale during the attn PSUM->SBUF copy.
  - Projection biases: per-partition ACT bias during PSUM->SBUF copies,
    or K=1 rank-1 matmul chunks for free-axis biases.
"""

import sys

for _p in ("/opt/trn_rl_repo",):
    if _p not in sys.path:
        sys.path.append(_p)

import numpy as np
import ml_dtypes

import concourse.bass as bass
import concourse.mybir as mybir
import concourse.tile as tile
from concourse import bacc
from concourse.bass_utils import run_bass_kernel_spmd

B, T, C, H, HS = 128, 256, 384, 6, 64
DFF = 4 * C
EPS = 1e-5
NCORES = 8
BL = B // NCORES          # batches per core (16)
NTOK = BL * T             # tokens per core (4096)
P = 128
CK = C // P               # channel chunks (3)
FK = DFF // P             # ffn chunks (12)
ST = 512                  # supertile tokens (2 batches)
NST = NTOK // ST          # supertiles per core (8)
NTT = ST // P             # token tiles per supertile (4)

F32 = mybir.dt.float32
BF16 = mybir.dt.bfloat16
AF = mybir.ActivationFunctionType
ALU = mybir.AluOpType
BF = ml_dtypes.bfloat16


def _layernorm_to_feature_major(nc, pools, x_tt, dst_T, identity, eps_tile):
    """LN token-major tiles of a supertile -> bf16 feature-major dst_T."""
    small, ps_tr, hn_pool = pools["small"], pools["ps_tr"], pools["hn"]
    for tt in range(NTT):
        xt = x_tt[:, tt, :]
        stats = small.tile([P, 6], F32, tag="stats")
        nc.vector.bn_stats(out=stats, in_=xt)
        mv = small.tile([P, 2], F32, tag="mv")
        nc.vector.bn_aggr(out=mv, in_=stats)
        # rstd = sqrt(1/(var+eps))
        iv = small.tile([P, 1], F32, tag="iv")
        nc.vector.tensor_scalar_add(out=iv, in0=mv[:, 1:2], scalar1=eps_tile)
        nc.vector.reciprocal(out=iv, in_=iv)
        rstd = small.tile([P, 1], F32, tag="rstd")
        nc.scalar.activation(out=rstd, in_=iv, func=AF.Sqrt)
        hn = hn_pool.tile([P, C], BF16, tag="hn")
        nc.vector.tensor_scalar(
            out=hn, in0=xt, scalar1=mv[:, 0:1], scalar2=rstd,
            op0=ALU.subtract, op1=ALU.mult,
        )
        for k in range(CK):
            ps = ps_tr.tile([P, P], BF16, tag="tr")
            nc.tensor.transpose(ps, hn[:, k * P:(k + 1) * P], identity)
            nc.vector.tensor_copy(out=dst_T[:, k, tt * P:(tt + 1) * P], in_=ps)


def build_transformer(nc):
    xs = nc.dram_tensor("xs", [NTOK, C], F32, kind="ExternalInput").ap()
    wq = nc.dram_tensor("wq", [CK, P, C], BF16, kind="ExternalInput").ap()
    wk = nc.dram_tensor("wk", [CK, P, C], BF16, kind="ExternalInput").ap()
    wv = nc.dram_tensor("wv", [CK, P, C], BF16, kind="ExternalInput").ap()
    wo = nc.dram_tensor("wo", [CK, P, C], BF16, kind="ExternalInput").ap()
    w1 = nc.dram_tensor("w1", [CK, P, DFF], BF16, kind="ExternalInput").ap()
    w2 = nc.dram_tensor("w2", [FK, P, C], BF16, kind="ExternalInput").ap()
    bq = nc.dram_tensor("bq", [C], F32, kind="ExternalInput").ap()
    bk = nc.dram_tensor("bk", [C], F32, kind="ExternalInput").ap()
    bv = nc.dram_tensor("bv", [C], BF16, kind="ExternalInput").ap()
    bo = nc.dram_tensor("bo", [C], BF16, kind="ExternalInput").ap()
    b1 = nc.dram_tensor("b1", [DFF], F32, kind="ExternalInput").ap()
    b2 = nc.dram_tensor("b2", [C], BF16, kind="ExternalInput").ap()
    ident = nc.dram_tensor("ident", [P, P], BF16, kind="ExternalInput").ap()
    onesr = nc.dram_tensor("onesr", [1, P], BF16, kind="ExternalInput").ap()
    out = nc.dram_tensor("out", [NTOK, C], F32, kind="ExternalOutput").ap()

    from contextlib import ExitStack
    with tile.TileContext(nc) as tc, ExitStack() as ctx:
        const = ctx.enter_context(tc.tile_pool(name="const", bufs=1))
        io_pool = ctx.enter_context(tc.tile_pool(name="io", bufs=2))
        act_pool = ctx.enter_context(tc.tile_pool(name="act", bufs=2))
        hn_pool = ctx.enter_context(tc.tile_pool(name="hn", bufs=3))
        wei_pool = ctx.enter_context(tc.tile_pool(name="wei", bufs=8))
        small = ctx.enter_context(tc.tile_pool(name="small", bufs=6))
        rc_pool = ctx.enter_context(tc.tile_pool(name="rc", bufs=6))
        ps_tr = ctx.enter_context(tc.tile_pool(name="ps_tr", bufs=2, space="PSUM"))
        ps_mm = ctx.enter_context(tc.tile_pool(name="ps_mm", bufs=3, space="PSUM"))
        ps_big = ctx.enter_context(tc.tile_pool(name="ps_big", bufs=1, space="PSUM"))
        ps_attn = ctx.enter_context(tc.tile_pool(name="ps_attn", bufs=2, space="PSUM"))
        pools = {"small": small, "hn": hn_pool, "ps_tr": ps_tr}

        # ---- persistent constants ----
        wq_sb = const.tile([P, CK, C], BF16)
        nc.sync.dma_start(out=wq_sb, in_=wq.rearrange("k p d -> p k d"))
        wk_sb = const.tile([P, CK, C], BF16)
        nc.sync.dma_start(out=wk_sb, in_=wk.rearrange("k p d -> p k d"))
        wv_sb = const.tile([P, CK, C], BF16)
        nc.sync.dma_start(out=wv_sb, in_=wv.rearrange("k p d -> p k d"))
        wo_sb = const.tile([P, CK, C], BF16)
        nc.sync.dma_start(out=wo_sb, in_=wo.rearrange("k p d -> p k d"))
        w1_sb = const.tile([P, CK, DFF], BF16)
        nc.sync.dma_start(out=w1_sb, in_=w1.rearrange("k p d -> p k d"))
        w2_sb = const.tile([P, FK, C], BF16)
        nc.sync.dma_start(out=w2_sb, in_=w2.rearrange("k p d -> p k d"))
        bq_sb = const.tile([P, CK], F32)
        nc.sync.dma_start(out=bq_sb, in_=bq.rearrange("(k p) -> p k", p=P))
        bk_sb = const.tile([P, CK], F32)
        nc.sync.dma_start(out=bk_sb, in_=bk.rearrange("(k p) -> p k", p=P))
        b1_sb = const.tile([P, FK], F32)
        nc.sync.dma_start(out=b1_sb, in_=b1.rearrange("(f p) -> p f", p=P))
        bv_row = const.tile([1, C], BF16)
        nc.sync.dma_start(out=bv_row, in_=bv.rearrange("(a d) -> a d", a=1))
        bo_row = const.tile([1, C], BF16)
        nc.sync.dma_start(out=bo_row, in_=bo.rearrange("(a d) -> a d", a=1))
        b2_row = const.tile([1, C], BF16)
        nc.sync.dma_start(out=b2_row, in_=b2.rearrange("(a d) -> a d", a=1))
        ones1 = const.tile([1, P], BF16)
        nc.sync.dma_start(out=ones1, in_=onesr)
        ones_bf = const.tile([P, 1], BF16)
        nc.vector.memset(ones_bf, 1.0)
        identity = const.tile([P, P], BF16)
        nc.sync.dma_start(out=identity, in_=ident)
        eps_tile = const.tile([P, 1], F32)
        nc.vector.memset(eps_tile, EPS)

        def load_and_ln1(g):
            """DMA x supertile g and LayerNorm it into a fresh h1T."""
            t0 = g * ST
            xa = io_pool.tile([P, NTT, C], F32, tag="xa", name=f"xa{g}")
            nc.sync.dma_start(
                out=xa, in_=xs[t0:t0 + ST, :].rearrange("(tt p) c -> p tt c", p=P))
            h1T = act_pool.tile([P, CK, ST], BF16, tag="h1T", name=f"h1T{g}")
            _layernorm_to_feature_major(nc, pools, xa, h1T, identity, eps_tile)
            return xa, h1T

        nxt = load_and_ln1(0)
        for g in range(NST):
            t0 = g * ST
            xa, h1T = nxt

            # ---- QKV projections (N=512) ----
            QT = act_pool.tile([P, CK, ST], BF16, tag="QT")
            KT = act_pool.tile([P, CK, ST], BF16, tag="KT")
            for m in range(CK):
                psq = ps_mm.tile([P, ST], F32, tag="mm")
                for k in range(CK):
                    nc.tensor.matmul(
                        psq, wq_sb[:, k, m * P:(m + 1) * P], h1T[:, k, :],
                        start=(k == 0), stop=(k == CK - 1))
                nc.vector.tensor_scalar_add(
                    out=QT[:, m, :], in0=psq, scalar1=bq_sb[:, m:m + 1])
                psk = ps_mm.tile([P, ST], F32, tag="mm")
                for k in range(CK):
                    nc.tensor.matmul(
                        psk, wk_sb[:, k, m * P:(m + 1) * P], h1T[:, k, :],
                        start=(k == 0), stop=(k == CK - 1))
                nc.vector.tensor_scalar_add(
                    out=KT[:, m, :], in0=psk, scalar1=bk_sb[:, m:m + 1])
            # V token-major, augmented with a ones column per head (65-wide
            # head slices) so attnV matmuls also produce the softmax row sums
            Vtm = act_pool.tile([P, NTT, H * 65], BF16, tag="Vtm")
            for tt in range(NTT):
                psv = ps_big.tile([P, C], F32, tag="big")
                for k in range(CK):
                    nc.tensor.matmul(
                        psv, h1T[:, k, tt * P:(tt + 1) * P], wv_sb[:, k, :],
                        start=(k == 0), stop=False)
                nc.tensor.matmul(psv, ones1, bv_row, start=False, stop=True)
                vview = Vtm[:, tt, :].rearrange("p (h e) -> p h e", e=65)
                nc.vector.tensor_copy(
                    out=vview[:, :, 0:HS],
                    in_=psv.rearrange("p (h e) -> p h e", e=HS))
                nc.gpsimd.memset(vview[:, :, HS:65], 1.0)

            # hoist next supertile's load + LN1 here: its DVE/ACT chain
            # overlaps attention's latency chains, and its finished h1T lets
            # the scheduler pull QKV(g+1) matmuls into the LN2(g) PE gap.
            if g + 1 < NST:
                nxt = load_and_ln1(g + 1)

            # ---- attention per batch ----
            attn_sb = hn_pool.tile([P, NTT, C], BF16, tag="attn_sb")
            for b2 in range(2):
                co = b2 * T          # column offset of this batch in supertile
                vo = b2 * 2          # Vtm token-tile offset
                attn_ps = [
                    ps_attn.tile([P, H * 65], F32, tag="attn",
                                 name=f"aps{g}_{b2}_{tt}")
                    for tt in range(2)
                ]
                weiTs = [
                    wei_pool.tile([P, 2, T], BF16, tag="weiT", name=f"w{g}_{b2}_{h}")
                    for h in range(H)
                ]
                # pass 1: scores + exp + mask. Heads are emitted in
                # even/odd pairs whose lhsT base partitions are 0 and 64, so
                # the two K=64 matmuls land on disjoint PE row groups and
                # run concurrently.
                for hp in range(H // 2):
                    h0, h1 = 2 * hp, 2 * hp + 1
                    q0 = QT[0:HS, hp, co:co + T]
                    k0 = KT[0:HS, hp, co:co + T]
                    q1 = QT[HS:2 * HS, hp, co:co + T]
                    k1 = KT[HS:2 * HS, hp, co:co + T]
                    ps0 = ps_mm.tile([P, ST], F32, tag="mm", name=f"s{g}{b2}{hp}0")
                    ps1 = ps_mm.tile([P, ST], F32, tag="mm", name=f"s{g}{b2}{hp}1")
                    nc.tensor.matmul(ps0[:, 0:T], k0[:, 0:P], q0,
                                     start=True, stop=True)
                    nc.tensor.matmul(ps1[:, 0:T], k1[:, 0:P], q1,
                                     start=True, stop=True)
                    nc.tensor.matmul(ps0[:, T:T + P], k0[:, P:], q0[:, P:],
                                     start=True, stop=True)
                    nc.tensor.matmul(ps1[:, T:T + P], k1[:, P:], q1[:, P:],
                                     start=True, stop=True)
                    for h, pss in ((h0, ps0), (h1, ps1)):
                        weiT = weiTs[h]
                        nc.scalar.activation(
                            out=weiT[:, 0, :], in_=pss[:, 0:T], func=AF.Exp,
                            scale=HS ** -0.5)
                        nc.scalar.activation(
                            out=weiT[:, 1, P:], in_=pss[:, T:T + P], func=AF.Exp,
                            scale=HS ** -0.5)
                        # causal mask on diagonal blocks: keep where t >= s
                        nc.gpsimd.affine_select(
                            out=weiT[:, 0, 0:P], in_=weiT[:, 0, 0:P],
                            compare_op=ALU.is_ge, fill=0.0, base=0,
                            pattern=[[1, P]], channel_multiplier=-1)
                        nc.gpsimd.affine_select(
                            out=weiT[:, 1, P:], in_=weiT[:, 1, P:],
                            compare_op=ALU.is_ge, fill=0.0, base=0,
                            pattern=[[1, P]], channel_multiplier=-1)
                # pass 2: attnV + row sums in one matmul per (s-chunk,
                # t-tile); rhs is the 65-wide augmented V head slice
                for h in range(H):
                    weiT = weiTs[h]
                    for tt in range(2):
                        dst = attn_ps[tt][:, h * 65:(h + 1) * 65]
                        if tt == 0:
                            nc.tensor.matmul(
                                dst, weiT[:, 0, 0:P],
                                Vtm[:, vo, h * 65:(h + 1) * 65],
                                start=True, stop=True)
                        else:
                            nc.tensor.matmul(
                                dst, weiT[:, 0, P:],
                                Vtm[:, vo, h * 65:(h + 1) * 65],
                                start=True, stop=False)
                            nc.tensor.matmul(
                                dst, weiT[:, 1, P:],
                                Vtm[:, vo + 1, h * 65:(h + 1) * 65],
                                start=False, stop=True)
                # one reciprocal over the 6 interleaved row sums, then
                # normalize via the per-partition ACT scale during the copy
                for tt in range(2):
                    aview = attn_ps[tt].rearrange("p (h e) -> p h e", e=65)
                    rc6 = rc_pool.tile([P, H], F32, tag="rc")
                    nc.vector.reciprocal(out=rc6, in_=aview[:, :, HS])
                    for h in range(H):
                        nc.scalar.activation(
                            out=attn_sb[:, vo + tt, h * HS:(h + 1) * HS],
                            in_=aview[:, h, 0:HS],
                            func=AF.Identity, scale=rc6[:, h:h + 1])

            # ---- transpose attn to feature-major; Wo + residual ----
            attn_T = act_pool.tile([P, CK, ST], BF16, tag="attnT")
            for tt in range(NTT):
                for k in range(CK):
                    pst = ps_tr.tile([P, P], BF16, tag="tr")
                    nc.tensor.transpose(
                        pst, attn_sb[:, tt, k * P:(k + 1) * P], identity)
                    nc.vector.tensor_copy(
                        out=attn_T[:, k, tt * P:(tt + 1) * P], in_=pst)
            xmid = io_pool.tile([P, NTT, C], F32, tag="xmid")
            for tt in range(NTT):
                pso = ps_big.tile([P, C], F32, tag="big")
                for k in range(CK):
                    nc.tensor.matmul(
                        pso, attn_T[:, k, tt * P:(tt + 1) * P], wo_sb[:, k, :],
                        start=(k == 0), stop=False)
                nc.tensor.matmul(pso, ones1, bo_row, start=False, stop=True)
                nc.vector.tensor_add(out=xmid[:, tt, :], in0=xa[:, tt, :], in1=pso)

            # ---- LN2 -> h2T ----
            h2T = act_pool.tile([P, CK, ST], BF16, tag="h2T")
            _layernorm_to_feature_major(nc, pools, xmid, h2T, identity, eps_tile)

            # ---- FFN ----
            ff1T = act_pool.tile([P, FK, ST], BF16, tag="ff1T")
            for f in range(FK):
                psf = ps_mm.tile([P, ST], F32, tag="mm")
                for k in range(CK):
                    nc.tensor.matmul(
                        psf, w1_sb[:, k, f * P:(f + 1) * P], h2T[:, k, :],
                        start=(k == 0), stop=(k == CK - 1))
                nc.scalar.activation(
                    out=ff1T[:, f, :], in_=psf, func=AF.Relu, bias=b1_sb[:, f:f + 1])
            for tt in range(NTT):
                ps2 = ps_big.tile([P, C], F32, tag="big")
                for f in range(FK):
                    nc.tensor.matmul(
                        ps2, ff1T[:, f, tt * P:(tt + 1) * P], w2_sb[:, f, :],
                        start=(f == 0), stop=False)
                nc.tensor.matmul(ps2, ones1, b2_row, start=False, stop=True)
                yt = io_pool.tile([P, C], F32, tag="yt")
                nc.vector.tensor_add(out=yt, in0=xmid[:, tt, :], in1=ps2)
                nc.sync.dma_start(
                    out=out[t0 + tt * P: t0 + (tt + 1) * P, :], in_=yt)
    return nc


_NC_CACHE = {}


def get_nc():
    if "nc" not in _NC_CACHE:
        nc = bacc.Bacc(
            "TRN2", target_bir_lowering=False, debug=False, num_devices=NCORES)
        build_transformer(nc)
        nc.compile()
        _NC_CACHE["nc"] = nc
    return _NC_CACHE["nc"]


def prep_inputs(x, Wq, Wk, Wv, Wo, bo, W1, b1, W2, b2, g1, be1, g2, be2):
    """Host-side exact folding of LN affine params into weights/biases, plus
    layout packing and bf16 casts."""
    f32 = np.float32
    x = np.asarray(x, f32)
    Wq2 = np.asarray(Wq, f32).transpose(1, 0, 2).reshape(C, C)
    Wk2 = np.asarray(Wk, f32).transpose(1, 0, 2).reshape(C, C)
    Wv2 = np.asarray(Wv, f32).transpose(1, 0, 2).reshape(C, C)
    g1 = np.asarray(g1, f32)
    be1 = np.asarray(be1, f32)
    g2 = np.asarray(g2, f32)
    be2 = np.asarray(be2, f32)
    bf = lambda a: np.ascontiguousarray(np.asarray(a, f32)).astype(BF)
    shared = {
        "wq": bf((g1[:, None] * Wq2).reshape(CK, P, C)),
        "wk": bf((g1[:, None] * Wk2).reshape(CK, P, C)),
        "wv": bf((g1[:, None] * Wv2).reshape(CK, P, C)),
        "wo": bf(np.asarray(Wo, f32).reshape(CK, P, C)),
        "w1": bf((g2[:, None] * np.asarray(W1, f32)).reshape(CK, P, DFF)),
        "w2": bf(np.asarray(W2, f32).reshape(FK, P, C)),
        "bq": np.ascontiguousarray(be1 @ Wq2),
        "bk": np.ascontiguousarray(be1 @ Wk2),
        "bv": bf(be1 @ Wv2),
        "bo": bf(np.asarray(bo, f32)),
        "b1": np.ascontiguousarray(be2 @ np.asarray(W1, f32) + np.asarray(b1, f32)),
        "b2": bf(np.asarray(b2, f32)),
        "ident": np.eye(P, dtype=f32).astype(BF),
        "onesr": np.ones((1, P), dtype=f32).astype(BF),
    }
    shards = [
        np.ascontiguousarray(x[i * BL:(i + 1) * BL].reshape(NTOK, C))
        for i in range(NCORES)
    ]
    return shared, shards


def run_on_device(nc, shared, shards, trace=False, **kwargs):
    in_maps = [dict(shared, xs=shards[i]) for i in range(NCORES)]
    return run_bass_kernel_spmd(
        nc, in_maps, core_ids=list(range(NCORES)), trace=trace, **kwargs)


def kernel(**inputs):
    nc = get_nc()
    shared, shards = prep_inputs(**inputs)
    res = run_on_device(nc, shared, shards, trace=False)
    out = np.concatenate(
        [res.results[i]["out"].reshape(BL, T, C) for i in range(NCORES)], axis=0)
    return out.astype(np.float32)


# revision 15
# speedup vs baseline: 1.8841x; 1.0717x over previous
"""Trainium2 Bass kernel for a pre-norm transformer block (causal MHA + FFN).

Sharding: pure data-parallel over batch B=128 across 8 NeuronCores
(16 batches/core). No collectives.

v2 layout (per core, 4096 tokens processed as 8 supertiles of 512 tokens
= 2 batches each):
  - All TensorE operands are bf16 (fast weight load + 1 cycle/row at any
    free dim); PSUM accumulation and residual adds stay fp32.
  - LayerNorm token-major via bn_stats; rstd = Sqrt(reciprocal(var+eps))
    (DVE reciprocal + ACT Sqrt); normalized output written bf16 and
    PE-transposed to feature-major [c, t] supertile operands.
  - QKV projections at N=512; attention per batch: transposed scores
    [s, t] per head, exp on ScalarE -> bf16 probs, causal mask via
    gpsimd.affine_select, per-head row sums matmul'd into one [128, 6]
    psum column each -> single reciprocal, normalization applied as the
    per-partition ACT scale during the attn PSUM->SBUF copy.
  - Projection biases: per-partition ACT bias during PSUM->SBUF copies,
    or K=1 rank-1 matmul chunks for free-axis biases.
"""

import sys

for _p in ("/opt/trn_rl_repo",):
    if _p not in sys.path:
        sys.path.append(_p)

import numpy as np
import ml_dtypes

import concourse.bass as bass
import concourse.mybir as mybir
import concourse.tile as tile
from concourse import bacc
from concourse.bass_utils import run_bass_kernel_spmd

B, T, C, H, HS = 128, 256, 384, 6, 64
DFF = 4 * C
EPS = 1e-5
NCORES = 8
BL = B // NCORES          # batches per core (16)
NTOK = BL * T             # tokens per core (4096)
P = 128
CK = C // P               # channel chunks (3)
FK = DFF // P             # ffn chunks (12)
ST = 512                  # supertile tokens (2 batches)
NST = NTOK // ST          # supertiles per core (8)
NTT = ST // P             # token tiles per supertile (4)

F32 = mybir.dt.float32
BF16 = mybir.dt.bfloat16
AF = mybir.ActivationFunctionType
ALU = mybir.AluOpType
BF = ml_dtypes.bfloat16


def _layernorm_to_feature_major(nc, pools, x_tt, dst_T, identity, eps_tile):
    """LN token-major tiles of a supertile -> bf16 feature-major dst_T."""
    small, ps_tr, hn_pool = pools["small"], pools["ps_tr"], pools["hn"]
    for tt in range(NTT):
        xt = x_tt[:, tt, :]
        stats = small.tile([P, 6], F32, tag="stats")
        nc.vector.bn_stats(out=stats, in_=xt)
        mv = small.tile([P, 2], F32, tag="mv")
        nc.vector.bn_aggr(out=mv, in_=stats)
        # rstd = sqrt(1/(var+eps))
        iv = small.tile([P, 1], F32, tag="iv")
        nc.vector.tensor_scalar_add(out=iv, in0=mv[:, 1:2], scalar1=eps_tile)
        nc.vector.reciprocal(out=iv, in_=iv)
        rstd = small.tile([P, 1], F32, tag="rstd")
        nc.scalar.activation(out=rstd, in_=iv, func=AF.Sqrt)
        hn = hn_pool.tile([P, C], BF16, tag="hn")
        nc.vector.tensor_scalar(
            out=hn, in0=xt, scalar1=mv[:, 0:1], scalar2=rstd,
            op0=ALU.subtract, op1=ALU.mult,
        )
        for k in range(CK):
            ps = ps_tr.tile([P, P], BF16, tag="tr")
            nc.tensor.transpose(ps, hn[:, k * P:(k + 1) * P], identity)
            nc.vector.tensor_copy(out=dst_T[:, k, tt * P:(tt + 1) * P], in_=ps)


def build_transformer(nc):
    xs = nc.dram_tensor("xs", [NTOK, C], F32, kind="ExternalInput").ap()
    wq = nc.dram_tensor("wq", [CK, P, C], BF16, kind="ExternalInput").ap()
    wk = nc.dram_tensor("wk", [CK, P, C], BF16, kind="ExternalInput").ap()
    wv = nc.dram_tensor("wv", [CK, P, C], BF16, kind="ExternalInput").ap()
    wo = nc.dram_tensor("wo", [CK, P, C], BF16, kind="ExternalInput").ap()
    w1 = nc.dram_tensor("w1", [CK, P, DFF], BF16, kind="ExternalInput").ap()
    w2 = nc.dram_tensor("w2", [FK, P, C], BF16, kind="ExternalInput").ap()
    bq = nc.dram_tensor("bq", [C], F32, kind="ExternalInput").ap()
    bk = nc.dram_tensor("bk", [C], F32, kind="ExternalInput").ap()
    bv = nc.dram_tensor("bv", [C], BF16, kind="ExternalInput").ap()
    bo = nc.dram_tensor("bo", [C], BF16, kind="ExternalInput").ap()
    b1 = nc.dram_tensor("b1", [DFF], F32, kind="ExternalInput").ap()
    b2 = nc.dram_tensor("b2", [C], BF16, kind="ExternalInput").ap()
    ident = nc.dram_tensor("ident", [P, P], BF16, kind="ExternalInput").ap()
    onesr = nc.dram_tensor("onesr", [1, P], BF16, kind="ExternalInput").ap()
    out = nc.dram_tensor("out", [NTOK, C], F32, kind="ExternalOutput").ap()

    from contextlib import ExitStack
    with tile.TileContext(nc) as tc, ExitStack() as ctx:
        const = ctx.enter_context(tc.tile_pool(name="const", bufs=1))
        io_pool = ctx.enter_context(tc.tile_pool(name="io", bufs=2))
        act_pool = ctx.enter_context(tc.tile_pool(name="act", bufs=2))
        hn_pool = ctx.enter_context(tc.tile_pool(name="hn", bufs=3))
        wei_pool = ctx.enter_context(tc.tile_pool(name="wei", bufs=8))
        small = ctx.enter_context(tc.tile_pool(name="small", bufs=6))
        rc_pool = ctx.enter_context(tc.tile_pool(name="rc", bufs=6))
        ps_tr = ctx.enter_context(tc.tile_pool(name="ps_tr", bufs=2, space="PSUM"))
        ps_mm = ctx.enter_context(tc.tile_pool(name="ps_mm", bufs=3, space="PSUM"))
        ps_big = ctx.enter_context(tc.tile_pool(name="ps_big", bufs=1, space="PSUM"))
        ps_attn = ctx.enter_context(tc.tile_pool(name="ps_attn", bufs=2, space="PSUM"))
        pools = {"small": small, "hn": hn_pool, "ps_tr": ps_tr}

        # ---- persistent constants ----
        wq_sb = const.tile([P, CK, C], BF16)
        nc.sync.dma_start(out=wq_sb, in_=wq.rearrange("k p d -> p k d"))
        wk_sb = const.tile([P, CK, C], BF16)
        nc.sync.dma_start(out=wk_sb, in_=wk.rearrange("k p d -> p k d"))
        wv_sb = const.tile([P, CK, C], BF16)
        nc.sync.dma_start(out=wv_sb, in_=wv.rearrange("k p d -> p k d"))
        wo_sb = const.tile([P, CK, C], BF16)
        nc.sync.dma_start(out=wo_sb, in_=wo.rearrange("k p d -> p k d"))
        w1_sb = const.tile([P, CK, DFF], BF16)
        nc.sync.dma_start(out=w1_sb, in_=w1.rearrange("k p d -> p k d"))
        w2_sb = const.tile([P, FK, C], BF16)
        nc.sync.dma_start(out=w2_sb, in_=w2.rearrange("k p d -> p k d"))
        bq_sb = const.tile([P, CK], F32)
        nc.sync.dma_start(out=bq_sb, in_=bq.rearrange("(k p) -> p k", p=P))
        bk_sb = const.tile([P, CK], F32)
        nc.sync.dma_start(out=bk_sb, in_=bk.rearrange("(k p) -> p k", p=P))
        b1_sb = const.tile([P, FK], F32)
        nc.sync.dma_start(out=b1_sb, in_=b1.rearrange("(f p) -> p f", p=P))
        bv_row = const.tile([1, C], BF16)
        nc.sync.dma_start(out=bv_row, in_=bv.rearrange("(a d) -> a d", a=1))
        bo_row = const.tile([1, C], BF16)
        nc.sync.dma_start(out=bo_row, in_=bo.rearrange("(a d) -> a d", a=1))
        b2_row = const.tile([1, C], BF16)
        nc.sync.dma_start(out=b2_row, in_=b2.rearrange("(a d) -> a d", a=1))
        ones1 = const.tile([1, P], BF16)
        nc.sync.dma_start(out=ones1, in_=onesr)
        ones_bf = const.tile([P, 1], BF16)
        nc.vector.memset(ones_bf, 1.0)
        identity = const.tile([P, P], BF16)
        nc.sync.dma_start(out=identity, in_=ident)
        eps_tile = const.tile([P, 1], F32)
        nc.vector.memset(eps_tile, EPS)

        def load_and_ln1(g):
            """DMA x supertile g and LayerNorm it into a fresh h1T."""
            t0 = g * ST
            xa = io_pool.tile([P, NTT, C], F32, tag="xa", name=f"xa{g}")
            nc.sync.dma_start(
                out=xa, in_=xs[t0:t0 + ST, :].rearrange("(tt p) c -> p tt c", p=P))
            h1T = act_pool.tile([P, CK, ST], BF16, tag="h1T", name=f"h1T{g}")
            _layernorm_to_feature_major(nc, pools, xa, h1T, identity, eps_tile)
            return xa, h1T

        def emit_ffn(h2T, xmid, t0):
            ff1T = act_pool.tile([P, FK, ST], BF16, tag="ff1T")
            for f in range(FK):
                psf = ps_mm.tile([P, ST], F32, tag="mm")
                for k in range(CK):
                    nc.tensor.matmul(
                        psf, w1_sb[:, k, f * P:(f + 1) * P], h2T[:, k, :],
                        start=(k == 0), stop=(k == CK - 1))
                nc.scalar.activation(
                    out=ff1T[:, f, :], in_=psf, func=AF.Relu, bias=b1_sb[:, f:f + 1])
            for tt in range(NTT):
                ps2 = ps_big.tile([P, C], F32, tag="big")
                for f in range(FK):
                    nc.tensor.matmul(
                        ps2, ff1T[:, f, tt * P:(tt + 1) * P], w2_sb[:, f, :],
                        start=(f == 0), stop=False)
                nc.tensor.matmul(ps2, ones1, b2_row, start=False, stop=True)
                yt = io_pool.tile([P, C], F32, tag="yt")
                nc.vector.tensor_add(out=yt, in0=xmid[:, tt, :], in1=ps2)
                nc.sync.dma_start(
                    out=out[t0 + tt * P: t0 + (tt + 1) * P, :], in_=yt)

        nxt = load_and_ln1(0)
        pend_ffn = None
        for g in range(NST):
            t0 = g * ST
            xa, h1T = nxt

            # ---- QKV projections (N=512) ----
            QT = act_pool.tile([P, CK, ST], BF16, tag="QT")
            KT = act_pool.tile([P, CK, ST], BF16, tag="KT")
            for m in range(CK):
                psq = ps_mm.tile([P, ST], F32, tag="mm")
                for k in range(CK):
                    nc.tensor.matmul(
                        psq, wq_sb[:, k, m * P:(m + 1) * P], h1T[:, k, :],
                        start=(k == 0), stop=(k == CK - 1))
                nc.vector.tensor_scalar_add(
                    out=QT[:, m, :], in0=psq, scalar1=bq_sb[:, m:m + 1])
                psk = ps_mm.tile([P, ST], F32, tag="mm")
                for k in range(CK):
                    nc.tensor.matmul(
                        psk, wk_sb[:, k, m * P:(m + 1) * P], h1T[:, k, :],
                        start=(k == 0), stop=(k == CK - 1))
                nc.vector.tensor_scalar_add(
                    out=KT[:, m, :], in0=psk, scalar1=bk_sb[:, m:m + 1])
            # V token-major, augmented with a ones column per head (65-wide
            # head slices) so attnV matmuls also produce the softmax row sums
            Vtm = act_pool.tile([P, NTT, H * 65], BF16, tag="Vtm")
            for tt in range(NTT):
                psv = ps_big.tile([P, C], F32, tag="big")
                for k in range(CK):
                    nc.tensor.matmul(
                        psv, h1T[:, k, tt * P:(tt + 1) * P], wv_sb[:, k, :],
                        start=(k == 0), stop=False)
                nc.tensor.matmul(psv, ones1, bv_row, start=False, stop=True)
                vview = Vtm[:, tt, :].rearrange("p (h e) -> p h e", e=65)
                nc.vector.tensor_copy(
                    out=vview[:, :, 0:HS],
                    in_=psv.rearrange("p (h e) -> p h e", e=HS))
                nc.gpsimd.memset(vview[:, :, HS:65], 1.0)

            # hoist next supertile's load + LN1 here: its DVE/ACT chain
            # overlaps attention's latency chains, and its finished h1T lets
            # the scheduler pull QKV(g+1) matmuls into the LN2(g) PE gap.
            if g + 1 < NST:
                nxt = load_and_ln1(g + 1)

            # ---- attention per batch ----
            attn_sb = hn_pool.tile([P, NTT, C], BF16, tag="attn_sb")
            for b2 in range(2):
                co = b2 * T          # column offset of this batch in supertile
                vo = b2 * 2          # Vtm token-tile offset
                attn_ps = [
                    ps_attn.tile([P, H * 65], F32, tag="attn",
                                 name=f"aps{g}_{b2}_{tt}")
                    for tt in range(2)
                ]
                weiTs = [
                    wei_pool.tile([P, 2, T], BF16, tag="weiT", name=f"w{g}_{b2}_{h}")
                    for h in range(H)
                ]
                # pass 1: scores + exp + mask. Heads are emitted in
                # even/odd pairs whose lhsT base partitions are 0 and 64, so
                # the two K=64 matmuls land on disjoint PE row groups and
                # run concurrently.
                for hp in range(H // 2):
                    h0, h1 = 2 * hp, 2 * hp + 1
                    q0 = QT[0:HS, hp, co:co + T]
                    k0 = KT[0:HS, hp, co:co + T]
                    q1 = QT[HS:2 * HS, hp, co:co + T]
                    k1 = KT[HS:2 * HS, hp, co:co + T]
                    ps0 = ps_mm.tile([P, ST], F32, tag="mm", name=f"s{g}{b2}{hp}0")
                    ps1 = ps_mm.tile([P, ST], F32, tag="mm", name=f"s{g}{b2}{hp}1")
                    nc.tensor.matmul(ps0[:, 0:T], k0[:, 0:P], q0,
                                     start=True, stop=True)
                    nc.tensor.matmul(ps1[:, 0:T], k1[:, 0:P], q1,
                                     start=True, stop=True)
                    nc.tensor.matmul(ps0[:, T:T + P], k0[:, P:], q0[:, P:],
                                     start=True, stop=True)
                    nc.tensor.matmul(ps1[:, T:T + P], k1[:, P:], q1[:, P:],
                                     start=True, stop=True)
                    for h, pss in ((h0, ps0), (h1, ps1)):
                        weiT = weiTs[h]
                        nc.scalar.activation(
                            out=weiT[:, 0, :], in_=pss[:, 0:T], func=AF.Exp,
                            scale=HS ** -0.5)
                        nc.scalar.activation(
                            out=weiT[:, 1, P:], in_=pss[:, T:T + P], func=AF.Exp,
                            scale=HS ** -0.5)
                        # causal mask on diagonal blocks: keep where t >= s
                        nc.gpsimd.affine_select(
                            out=weiT[:, 0, 0:P], in_=weiT[:, 0, 0:P],
                            compare_op=ALU.is_ge, fill=0.0, base=0,
                            pattern=[[1, P]], channel_multiplier=-1)
                        nc.gpsimd.affine_select(
                            out=weiT[:, 1, P:], in_=weiT[:, 1, P:],
                            compare_op=ALU.is_ge, fill=0.0, base=0,
                            pattern=[[1, P]], channel_multiplier=-1)
                # pass 2: attnV + row sums in one matmul per (s-chunk,
                # t-tile); rhs is the 65-wide augmented V head slice
                for h in range(H):
                    weiT = weiTs[h]
                    for tt in range(2):
                        dst = attn_ps[tt][:, h * 65:(h + 1) * 65]
                        if tt == 0:
                            nc.tensor.matmul(
                                dst, weiT[:, 0, 0:P],
                                Vtm[:, vo, h * 65:(h + 1) * 65],
                                start=True, stop=True)
                        else:
                            nc.tensor.matmul(
                                dst, weiT[:, 0, P:],
                                Vtm[:, vo, h * 65:(h + 1) * 65],
                                start=True, stop=False)
                            nc.tensor.matmul(
                                dst, weiT[:, 1, P:],
                                Vtm[:, vo + 1, h * 65:(h + 1) * 65],
                                start=False, stop=True)
                # one reciprocal over the 6 interleaved row sums, then
                # normalize via the per-partition ACT scale during the copy
                for tt in range(2):
                    aview = attn_ps[tt].rearrange("p (h e) -> p h e", e=65)
                    rc6 = rc_pool.tile([P, H], F32, tag="rc")
                    nc.vector.reciprocal(out=rc6, in_=aview[:, :, HS])
                    for h in range(H):
                        nc.scalar.activation(
                            out=attn_sb[:, vo + tt, h * HS:(h + 1) * HS],
                            in_=aview[:, h, 0:HS],
                            func=AF.Identity, scale=rc6[:, h:h + 1])

            # delayed FFN of the previous supertile: dense PE filler for the
            # attention/Wo latency chains above
            if pend_ffn is not None:
                emit_ffn(*pend_ffn)

            # ---- transpose attn to feature-major; Wo + residual ----
            attn_T = act_pool.tile([P, CK, ST], BF16, tag="attnT")
            for tt in range(NTT):
                for k in range(CK):
                    pst = ps_tr.tile([P, P], BF16, tag="tr")
                    nc.tensor.transpose(
                        pst, attn_sb[:, tt, k * P:(k + 1) * P], identity)
                    nc.vector.tensor_copy(
                        out=attn_T[:, k, tt * P:(tt + 1) * P], in_=pst)
            xmid = io_pool.tile([P, NTT, C], F32, tag="xmid")
            for tt in range(NTT):
                pso = ps_big.tile([P, C], F32, tag="big")
                for k in range(CK):
                    nc.tensor.matmul(
                        pso, attn_T[:, k, tt * P:(tt + 1) * P], wo_sb[:, k, :],
                        start=(k == 0), stop=False)
                nc.tensor.matmul(pso, ones1, bo_row, start=False, stop=True)
                nc.vector.tensor_add(out=xmid[:, tt, :], in0=xa[:, tt, :], in1=pso)

            # ---- LN2 -> h2T ----
            h2T = act_pool.tile([P, CK, ST], BF16, tag="h2T")
            _layernorm_to_feature_major(nc, pools, xmid, h2T, identity, eps_tile)

            # FFN for THIS supertile is emitted one iteration later (see
            # emit_ffn below) so its dense matmuls can fill the PE during the
            # next supertile's attention latency chains.
            pend_ffn = (h2T, xmid, t0)
        emit_ffn(*pend_ffn)
    return nc


_NC_CACHE = {}


def get_nc():
    if "nc" not in _NC_CACHE:
        nc = bacc.Bacc(
            "TRN2", target_bir_lowering=False, debug=False, num_devices=NCORES)
        build_transformer(nc)
        nc.compile()
        _NC_CACHE["nc"] = nc
    return _NC_CACHE["nc"]


def prep_inputs(x, Wq, Wk, Wv, Wo, bo, W1, b1, W2, b2, g1, be1, g2, be2):
    """Host-side exact folding of LN affine params into weights/biases, plus
    layout packing and bf16 casts."""
    f32 = np.float32
    x = np.asarray(x, f32)
    Wq2 = np.asarray(Wq, f32).transpose(1, 0, 2).reshape(C, C)
    Wk2 = np.asarray(Wk, f32).transpose(1, 0, 2).reshape(C, C)
    Wv2 = np.asarray(Wv, f32).transpose(1, 0, 2).reshape(C, C)
    g1 = np.asarray(g1, f32)
    be1 = np.asarray(be1, f32)
    g2 = np.asarray(g2, f32)
    be2 = np.asarray(be2, f32)
    bf = lambda a: np.ascontiguousarray(np.asarray(a, f32)).astype(BF)
    shared = {
        "wq": bf((g1[:, None] * Wq2).reshape(CK, P, C)),
        "wk": bf((g1[:, None] * Wk2).reshape(CK, P, C)),
        "wv": bf((g1[:, None] * Wv2).reshape(CK, P, C)),
        "wo": bf(np.asarray(Wo, f32).reshape(CK, P, C)),
        "w1": bf((g2[:, None] * np.asarray(W1, f32)).reshape(CK, P, DFF)),
        "w2": bf(np.asarray(W2, f32).reshape(FK, P, C)),
        "bq": np.ascontiguousarray(be1 @ Wq2),
        "bk": np.ascontiguousarray(be1 @ Wk2),
        "bv": bf(be1 @ Wv2),
        "bo": bf(np.asarray(bo, f32)),
        "b1": np.ascontiguousarray(be2 @ np.asarray(W1, f32) + np.asarray(b1, f32)),
        "b2": bf(np.asarray(b2, f32)),
        "ident": np.eye(P, dtype=f32).astype(BF),
        "onesr": np.ones((1, P), dtype=f32).astype(BF),
    }
    shards = [
        np.ascontiguousarray(x[i * BL:(i + 1) * BL].reshape(NTOK, C))
        for i in range(NCORES)
    ]
    return shared, shards


def run_on_device(nc, shared, shards, trace=False, **kwargs):
    in_maps = [dict(shared, xs=shards[i]) for i in range(NCORES)]
    return run_bass_kernel_spmd(
        nc, in_maps, core_ids=list(range(NCORES)), trace=trace, **kwargs)


def kernel(**inputs):
    nc = get_nc()
    shared, shards = prep_inputs(**inputs)
    res = run_on_device(nc, shared, shards, trace=False)
    out = np.concatenate(
        [res.results[i]["out"].reshape(BL, T, C) for i in range(NCORES)], axis=0)
    return out.astype(np.float32)


# revision 16
# speedup vs baseline: 2.1457x; 1.1388x over previous
"""Trainium2 Bass kernel for a pre-norm transformer block (causal MHA + FFN).

Sharding: pure data-parallel over batch B=128 across 8 NeuronCores
(16 batches/core). No collectives.

v2 layout (per core, 4096 tokens processed as 8 supertiles of 512 tokens
= 2 batches each):
  - All TensorE operands are bf16 (fast weight load + 1 cycle/row at any
    free dim); PSUM accumulation and residual adds stay fp32.
  - LayerNorm token-major via bn_stats; rstd = Sqrt(reciprocal(var+eps))
    (DVE reciprocal + ACT Sqrt); normalized output written bf16 and
    PE-transposed to feature-major [c, t] supertile operands.
  - QKV projections at N=512; attention per batch: transposed scores
    [s, t] per head, exp on ScalarE -> bf16 probs, causal mask via
    gpsimd.affine_select, per-head row sums matmul'd into one [128, 6]
    psum column each -> single reciprocal, normalization applied as the
    per-partition ACT scale during the attn PSUM->SBUF copy.
  - Projection biases: per-partition ACT bias during PSUM->SBUF copies,
    or K=1 rank-1 matmul chunks for free-axis biases.
"""

import sys

for _p in ("/opt/trn_rl_repo",):
    if _p not in sys.path:
        sys.path.append(_p)

import numpy as np
import ml_dtypes

import concourse.bass as bass
import concourse.mybir as mybir
import concourse.tile as tile
from concourse import bacc
from concourse.bass_utils import run_bass_kernel_spmd

B, T, C, H, HS = 128, 256, 384, 6, 64
DFF = 4 * C
EPS = 1e-5
NCORES = 8
BL = B // NCORES          # batches per core (16)
NTOK = BL * T             # tokens per core (4096)
P = 128
CK = C // P               # channel chunks (3)
FK = DFF // P             # ffn chunks (12)
ST = 512                  # supertile tokens (2 batches)
NST = NTOK // ST          # supertiles per core (8)
NTT = ST // P             # token tiles per supertile (4)

F32 = mybir.dt.float32
BF16 = mybir.dt.bfloat16
AF = mybir.ActivationFunctionType
ALU = mybir.AluOpType
BF = ml_dtypes.bfloat16


def _layernorm_to_feature_major(nc, pools, x_tt, dst_T, identity, eps_tile):
    """LN token-major tiles of a supertile -> bf16 feature-major dst_T."""
    small, ps_tr, hn_pool = pools["small"], pools["ps_tr"], pools["hn"]
    for tt in range(NTT):
        xt = x_tt[:, tt, :]
        stats = small.tile([P, 6], F32, tag="stats")
        nc.vector.bn_stats(out=stats, in_=xt)
        mv = small.tile([P, 2], F32, tag="mv")
        nc.vector.bn_aggr(out=mv, in_=stats)
        # rstd = exp(-0.5*ln(var+eps)): Ln and Exp share one ACT table set
        lnv = small.tile([P, 1], F32, tag="lnv")
        nc.scalar.activation(out=lnv, in_=mv[:, 1:2], func=AF.Ln, bias=eps_tile)
        rstd = small.tile([P, 1], F32, tag="rstd")
        nc.scalar.activation(out=rstd, in_=lnv, func=AF.Exp, scale=-0.5)
        hn = hn_pool.tile([P, C], BF16, tag="hn")
        nc.vector.tensor_scalar(
            out=hn, in0=xt, scalar1=mv[:, 0:1], scalar2=rstd,
            op0=ALU.subtract, op1=ALU.mult,
        )
        for k in range(CK):
            ps = ps_tr.tile([P, P], BF16, tag="tr")
            nc.tensor.transpose(ps, hn[:, k * P:(k + 1) * P], identity)
            nc.vector.tensor_copy(out=dst_T[:, k, tt * P:(tt + 1) * P], in_=ps)


def build_transformer(nc):
    xs = nc.dram_tensor("xs", [NTOK, C], F32, kind="ExternalInput").ap()
    wq = nc.dram_tensor("wq", [CK, P, C], BF16, kind="ExternalInput").ap()
    wk = nc.dram_tensor("wk", [CK, P, C], BF16, kind="ExternalInput").ap()
    wv = nc.dram_tensor("wv", [CK, P, C], BF16, kind="ExternalInput").ap()
    wo = nc.dram_tensor("wo", [CK, P, C], BF16, kind="ExternalInput").ap()
    w1 = nc.dram_tensor("w1", [CK, P, DFF], BF16, kind="ExternalInput").ap()
    w2 = nc.dram_tensor("w2", [FK, P, C], BF16, kind="ExternalInput").ap()
    bq = nc.dram_tensor("bq", [C], F32, kind="ExternalInput").ap()
    bk = nc.dram_tensor("bk", [C], F32, kind="ExternalInput").ap()
    bv = nc.dram_tensor("bv", [C], BF16, kind="ExternalInput").ap()
    bo = nc.dram_tensor("bo", [C], BF16, kind="ExternalInput").ap()
    b1 = nc.dram_tensor("b1", [DFF], F32, kind="ExternalInput").ap()
    b2 = nc.dram_tensor("b2", [C], BF16, kind="ExternalInput").ap()
    ident = nc.dram_tensor("ident", [P, P], BF16, kind="ExternalInput").ap()
    onesr = nc.dram_tensor("onesr", [1, P], BF16, kind="ExternalInput").ap()
    out = nc.dram_tensor("out", [NTOK, C], F32, kind="ExternalOutput").ap()

    from contextlib import ExitStack
    with tile.TileContext(nc) as tc, ExitStack() as ctx:
        const = ctx.enter_context(tc.tile_pool(name="const", bufs=1))
        io_pool = ctx.enter_context(tc.tile_pool(name="io", bufs=2))
        act_pool = ctx.enter_context(tc.tile_pool(name="act", bufs=2))
        hn_pool = ctx.enter_context(tc.tile_pool(name="hn", bufs=3))
        wei_pool = ctx.enter_context(tc.tile_pool(name="wei", bufs=8))
        small = ctx.enter_context(tc.tile_pool(name="small", bufs=6))
        rc_pool = ctx.enter_context(tc.tile_pool(name="rc", bufs=6))
        ps_tr = ctx.enter_context(tc.tile_pool(name="ps_tr", bufs=2, space="PSUM"))
        ps_mm = ctx.enter_context(tc.tile_pool(name="ps_mm", bufs=3, space="PSUM"))
        ps_big = ctx.enter_context(tc.tile_pool(name="ps_big", bufs=1, space="PSUM"))
        ps_attn = ctx.enter_context(tc.tile_pool(name="ps_attn", bufs=2, space="PSUM"))
        pools = {"small": small, "hn": hn_pool, "ps_tr": ps_tr}

        # ---- persistent constants ----
        wq_sb = const.tile([P, CK, C], BF16)
        nc.sync.dma_start(out=wq_sb, in_=wq.rearrange("k p d -> p k d"))
        wk_sb = const.tile([P, CK, C], BF16)
        nc.sync.dma_start(out=wk_sb, in_=wk.rearrange("k p d -> p k d"))
        wv_sb = const.tile([P, CK, C], BF16)
        nc.sync.dma_start(out=wv_sb, in_=wv.rearrange("k p d -> p k d"))
        wo_sb = const.tile([P, CK, C], BF16)
        nc.sync.dma_start(out=wo_sb, in_=wo.rearrange("k p d -> p k d"))
        w1_sb = const.tile([P, CK, DFF], BF16)
        nc.sync.dma_start(out=w1_sb, in_=w1.rearrange("k p d -> p k d"))
        w2_sb = const.tile([P, FK, C], BF16)
        nc.sync.dma_start(out=w2_sb, in_=w2.rearrange("k p d -> p k d"))
        bq_sb = const.tile([P, CK], F32)
        nc.sync.dma_start(out=bq_sb, in_=bq.rearrange("(k p) -> p k", p=P))
        bk_sb = const.tile([P, CK], F32)
        nc.sync.dma_start(out=bk_sb, in_=bk.rearrange("(k p) -> p k", p=P))
        b1_sb = const.tile([P, FK], F32)
        nc.sync.dma_start(out=b1_sb, in_=b1.rearrange("(f p) -> p f", p=P))
        bv_row = const.tile([1, C], BF16)
        nc.sync.dma_start(out=bv_row, in_=bv.rearrange("(a d) -> a d", a=1))
        bo_row = const.tile([1, C], BF16)
        nc.sync.dma_start(out=bo_row, in_=bo.rearrange("(a d) -> a d", a=1))
        b2_row = const.tile([1, C], BF16)
        nc.sync.dma_start(out=b2_row, in_=b2.rearrange("(a d) -> a d", a=1))
        ones1 = const.tile([1, P], BF16)
        nc.sync.dma_start(out=ones1, in_=onesr)
        ones_bf = const.tile([P, 1], BF16)
        nc.vector.memset(ones_bf, 1.0)
        identity = const.tile([P, P], BF16)
        nc.sync.dma_start(out=identity, in_=ident)
        eps_tile = const.tile([P, 1], F32)
        nc.vector.memset(eps_tile, EPS)

        def load_and_ln1(g):
            """DMA x supertile g and LayerNorm it into a fresh h1T."""
            t0 = g * ST
            xa = io_pool.tile([P, NTT, C], F32, tag="xa", name=f"xa{g}")
            nc.sync.dma_start(
                out=xa, in_=xs[t0:t0 + ST, :].rearrange("(tt p) c -> p tt c", p=P))
            h1T = act_pool.tile([P, CK, ST], BF16, tag="h1T", name=f"h1T{g}")
            _layernorm_to_feature_major(nc, pools, xa, h1T, identity, eps_tile)
            return xa, h1T

        def emit_ffn(h2T, xmid, t0):
            ff1T = act_pool.tile([P, FK, ST], BF16, tag="ff1T")
            for f in range(FK):
                psf = ps_mm.tile([P, ST], F32, tag="mm")
                for k in range(CK):
                    nc.tensor.matmul(
                        psf, w1_sb[:, k, f * P:(f + 1) * P], h2T[:, k, :],
                        start=(k == 0), stop=(k == CK - 1))
                nc.scalar.activation(
                    out=ff1T[:, f, :], in_=psf, func=AF.Relu, bias=b1_sb[:, f:f + 1])
            for tt in range(NTT):
                ps2 = ps_big.tile([P, C], F32, tag="big")
                for f in range(FK):
                    nc.tensor.matmul(
                        ps2, ff1T[:, f, tt * P:(tt + 1) * P], w2_sb[:, f, :],
                        start=(f == 0), stop=False)
                nc.tensor.matmul(ps2, ones1, b2_row, start=False, stop=True)
                yt = io_pool.tile([P, C], F32, tag="yt")
                nc.vector.tensor_add(out=yt, in0=xmid[:, tt, :], in1=ps2)
                nc.sync.dma_start(
                    out=out[t0 + tt * P: t0 + (tt + 1) * P, :], in_=yt)

        nxt = load_and_ln1(0)
        pend_ffn = None
        for g in range(NST):
            t0 = g * ST
            xa, h1T = nxt

            # ---- QKV projections (N=512) ----
            QT = act_pool.tile([P, CK, ST], BF16, tag="QT")
            KT = act_pool.tile([P, CK, ST], BF16, tag="KT")
            for m in range(CK):
                psq = ps_mm.tile([P, ST], F32, tag="mm")
                for k in range(CK):
                    nc.tensor.matmul(
                        psq, wq_sb[:, k, m * P:(m + 1) * P], h1T[:, k, :],
                        start=(k == 0), stop=(k == CK - 1))
                nc.vector.tensor_scalar_add(
                    out=QT[:, m, :], in0=psq, scalar1=bq_sb[:, m:m + 1])
                psk = ps_mm.tile([P, ST], F32, tag="mm")
                for k in range(CK):
                    nc.tensor.matmul(
                        psk, wk_sb[:, k, m * P:(m + 1) * P], h1T[:, k, :],
                        start=(k == 0), stop=(k == CK - 1))
                nc.vector.tensor_scalar_add(
                    out=KT[:, m, :], in0=psk, scalar1=bk_sb[:, m:m + 1])
            # V token-major, augmented with a ones column per head (65-wide
            # head slices) so attnV matmuls also produce the softmax row sums
            Vtm = act_pool.tile([P, NTT, H * 65], BF16, tag="Vtm")
            for tt in range(NTT):
                psv = ps_big.tile([P, C], F32, tag="big")
                for k in range(CK):
                    nc.tensor.matmul(
                        psv, h1T[:, k, tt * P:(tt + 1) * P], wv_sb[:, k, :],
                        start=(k == 0), stop=False)
                nc.tensor.matmul(psv, ones1, bv_row, start=False, stop=True)
                vview = Vtm[:, tt, :].rearrange("p (h e) -> p h e", e=65)
                nc.vector.tensor_copy(
                    out=vview[:, :, 0:HS],
                    in_=psv.rearrange("p (h e) -> p h e", e=HS))
                nc.gpsimd.memset(vview[:, :, HS:65], 1.0)

            # hoist next supertile's load + LN1 here: its DVE/ACT chain
            # overlaps attention's latency chains, and its finished h1T lets
            # the scheduler pull QKV(g+1) matmuls into the LN2(g) PE gap.
            if g + 1 < NST:
                nxt = load_and_ln1(g + 1)

            # ---- attention per batch ----
            attn_sb = hn_pool.tile([P, NTT, C], BF16, tag="attn_sb")
            for b2 in range(2):
                co = b2 * T          # column offset of this batch in supertile
                vo = b2 * 2          # Vtm token-tile offset
                attn_ps = [
                    ps_attn.tile([P, H * 65], F32, tag="attn",
                                 name=f"aps{g}_{b2}_{tt}")
                    for tt in range(2)
                ]
                weiTs = [
                    wei_pool.tile([P, 2, T], BF16, tag="weiT", name=f"w{g}_{b2}_{h}")
                    for h in range(H)
                ]
                # pass 1: scores + exp + mask. Heads are emitted in
                # even/odd pairs whose lhsT base partitions are 0 and 64, so
                # the two K=64 matmuls land on disjoint PE row groups and
                # run concurrently.
                for hp in range(H // 2):
                    h0, h1 = 2 * hp, 2 * hp + 1
                    q0 = QT[0:HS, hp, co:co + T]
                    k0 = KT[0:HS, hp, co:co + T]
                    q1 = QT[HS:2 * HS, hp, co:co + T]
                    k1 = KT[HS:2 * HS, hp, co:co + T]
                    ps0 = ps_mm.tile([P, ST], F32, tag="mm", name=f"s{g}{b2}{hp}0")
                    ps1 = ps_mm.tile([P, ST], F32, tag="mm", name=f"s{g}{b2}{hp}1")
                    nc.tensor.matmul(ps0[:, 0:T], k0[:, 0:P], q0,
                                     start=True, stop=True)
                    nc.tensor.matmul(ps1[:, 0:T], k1[:, 0:P], q1,
                                     start=True, stop=True)
                    nc.tensor.matmul(ps0[:, T:T + P], k0[:, P:], q0[:, P:],
                                     start=True, stop=True)
                    nc.tensor.matmul(ps1[:, T:T + P], k1[:, P:], q1[:, P:],
                                     start=True, stop=True)
                    for h, pss in ((h0, ps0), (h1, ps1)):
                        weiT = weiTs[h]
                        nc.scalar.activation(
                            out=weiT[:, 0, :], in_=pss[:, 0:T], func=AF.Exp,
                            scale=HS ** -0.5)
                        nc.scalar.activation(
                            out=weiT[:, 1, P:], in_=pss[:, T:T + P], func=AF.Exp,
                            scale=HS ** -0.5)
                        # causal mask on diagonal blocks: keep where t >= s
                        nc.gpsimd.affine_select(
                            out=weiT[:, 0, 0:P], in_=weiT[:, 0, 0:P],
                            compare_op=ALU.is_ge, fill=0.0, base=0,
                            pattern=[[1, P]], channel_multiplier=-1)
                        nc.gpsimd.affine_select(
                            out=weiT[:, 1, P:], in_=weiT[:, 1, P:],
                            compare_op=ALU.is_ge, fill=0.0, base=0,
                            pattern=[[1, P]], channel_multiplier=-1)
                # pass 2: attnV + row sums in one matmul per (s-chunk,
                # t-tile); rhs is the 65-wide augmented V head slice
                for h in range(H):
                    weiT = weiTs[h]
                    for tt in range(2):
                        dst = attn_ps[tt][:, h * 65:(h + 1) * 65]
                        if tt == 0:
                            nc.tensor.matmul(
                                dst, weiT[:, 0, 0:P],
                                Vtm[:, vo, h * 65:(h + 1) * 65],
                                start=True, stop=True)
                        else:
                            nc.tensor.matmul(
                                dst, weiT[:, 0, P:],
                                Vtm[:, vo, h * 65:(h + 1) * 65],
                                start=True, stop=False)
                            nc.tensor.matmul(
                                dst, weiT[:, 1, P:],
                                Vtm[:, vo + 1, h * 65:(h + 1) * 65],
                                start=False, stop=True)
                # one reciprocal over the 6 interleaved row sums, then
                # normalize via the per-partition ACT scale during the copy
                for tt in range(2):
                    aview = attn_ps[tt].rearrange("p (h e) -> p h e", e=65)
                    rc6 = rc_pool.tile([P, H], F32, tag="rc")
                    nc.vector.reciprocal(out=rc6, in_=aview[:, :, HS])
                    for h in range(H):
                        nc.scalar.activation(
                            out=attn_sb[:, vo + tt, h * HS:(h + 1) * HS],
                            in_=aview[:, h, 0:HS],
                            func=AF.Identity, scale=rc6[:, h:h + 1])

            # delayed FFN of the previous supertile: dense PE filler for the
            # attention/Wo latency chains above
            if pend_ffn is not None:
                emit_ffn(*pend_ffn)

            # ---- transpose attn to feature-major; Wo + residual ----
            attn_T = act_pool.tile([P, CK, ST], BF16, tag="attnT")
            for tt in range(NTT):
                for k in range(CK):
                    pst = ps_tr.tile([P, P], BF16, tag="tr")
                    nc.tensor.transpose(
                        pst, attn_sb[:, tt, k * P:(k + 1) * P], identity)
                    nc.vector.tensor_copy(
                        out=attn_T[:, k, tt * P:(tt + 1) * P], in_=pst)
            xmid = io_pool.tile([P, NTT, C], F32, tag="xmid")
            for tt in range(NTT):
                pso = ps_big.tile([P, C], F32, tag="big")
                for k in range(CK):
                    nc.tensor.matmul(
                        pso, attn_T[:, k, tt * P:(tt + 1) * P], wo_sb[:, k, :],
                        start=(k == 0), stop=False)
                nc.tensor.matmul(pso, ones1, bo_row, start=False, stop=True)
                nc.vector.tensor_add(out=xmid[:, tt, :], in0=xa[:, tt, :], in1=pso)

            # ---- LN2 -> h2T ----
            h2T = act_pool.tile([P, CK, ST], BF16, tag="h2T")
            _layernorm_to_feature_major(nc, pools, xmid, h2T, identity, eps_tile)

            # FFN for THIS supertile is emitted one iteration later (see
            # emit_ffn below) so its dense matmuls can fill the PE during the
            # next supertile's attention latency chains.
            pend_ffn = (h2T, xmid, t0)
        emit_ffn(*pend_ffn)
    return nc


_NC_CACHE = {}


class _PinnedActBacc(bacc.Bacc):
    """Pin all ACT functions to the natural_log_exp_and_others table set.

    The kernel only uses Exp, Ln, Relu and Identity, all of which live in
    that one set; the default per-function greedy pick alternates between
    exp/sqrt/log sets and pays ~2.7us per switch. Blanking the other sets
    (indexes preserved) makes the fixpoint choose one set, loaded once.
    """

    def insert_act_table_loads(self):
        import concourse.mybir as _mb
        from concourse.hw_specs import get_activation_tables
        has_activation = any(
            isinstance(i, _mb.InstActivation)
            for b in self.main_func.blocks
            for i in b.instructions
        )
        if not has_activation:
            return
        keep = "natural_log_exp_and_others"
        tables = [
            (k, (v if k == keep else set()))
            for k, v in get_activation_tables(self.m.arch).items()
        ]
        bacc._bass_rust.insert_act_table_loads(self, tables)


def get_nc():
    if "nc" not in _NC_CACHE:
        nc = _PinnedActBacc(
            "TRN2", target_bir_lowering=False, debug=False, num_devices=NCORES)
        build_transformer(nc)
        nc.compile()
        _NC_CACHE["nc"] = nc
    return _NC_CACHE["nc"]


def prep_inputs(x, Wq, Wk, Wv, Wo, bo, W1, b1, W2, b2, g1, be1, g2, be2):
    """Host-side exact folding of LN affine params into weights/biases, plus
    layout packing and bf16 casts."""
    f32 = np.float32
    x = np.asarray(x, f32)
    Wq2 = np.asarray(Wq, f32).transpose(1, 0, 2).reshape(C, C)
    Wk2 = np.asarray(Wk, f32).transpose(1, 0, 2).reshape(C, C)
    Wv2 = np.asarray(Wv, f32).transpose(1, 0, 2).reshape(C, C)
    g1 = np.asarray(g1, f32)
    be1 = np.asarray(be1, f32)
    g2 = np.asarray(g2, f32)
    be2 = np.asarray(be2, f32)
    bf = lambda a: np.ascontiguousarray(np.asarray(a, f32)).astype(BF)
    shared = {
        "wq": bf((g1[:, None] * Wq2).reshape(CK, P, C)),
        "wk": bf((g1[:, None] * Wk2).reshape(CK, P, C)),
        "wv": bf((g1[:, None] * Wv2).reshape(CK, P, C)),
        "wo": bf(np.asarray(Wo, f32).reshape(CK, P, C)),
        "w1": bf((g2[:, None] * np.asarray(W1, f32)).reshape(CK, P, DFF)),
        "w2": bf(np.asarray(W2, f32).reshape(FK, P, C)),
        "bq": np.ascontiguousarray(be1 @ Wq2),
        "bk": np.ascontiguousarray(be1 @ Wk2),
        "bv": bf(be1 @ Wv2),
        "bo": bf(np.asarray(bo, f32)),
        "b1": np.ascontiguousarray(be2 @ np.asarray(W1, f32) + np.asarray(b1, f32)),
        "b2": bf(np.asarray(b2, f32)),
        "ident": np.eye(P, dtype=f32).astype(BF),
        "onesr": np.ones((1, P), dtype=f32).astype(BF),
    }
    shards = [
        np.ascontiguousarray(x[i * BL:(i + 1) * BL].reshape(NTOK, C))
        for i in range(NCORES)
    ]
    return shared, shards


def run_on_device(nc, shared, shards, trace=False, **kwargs):
    in_maps = [dict(shared, xs=shards[i]) for i in range(NCORES)]
    return run_bass_kernel_spmd(
        nc, in_maps, core_ids=list(range(NCORES)), trace=trace, **kwargs)


def kernel(**inputs):
    nc = get_nc()
    shared, shards = prep_inputs(**inputs)
    res = run_on_device(nc, shared, shards, trace=False)
    out = np.concatenate(
        [res.results[i]["out"].reshape(BL, T, C) for i in range(NCORES)], axis=0)
    return out.astype(np.float32)
